# revision 23
# baseline (speedup 1.0000x reference)
"""Trainium2 Bass kernel for nn_DualStateLinearAttention.

Reference math (B=2, S=2048, HID=2048, H=16, D=128):
    q = x @ Wq.T, k = x @ Wk.T, v = x @ Wv.T            (split into 16 heads)
    gk_j = clamp(log_sigmoid(x @ Wgj.T + bgj) / 16, min=-50)   j in {1,2}
    o_j  = GLA scan over S with per-key-dim decay exp(gk_j)
    out  = (softmax(alpha)[0] * o1 + softmax(alpha)[1] * o2) @ Wo.T

Strategy (8 NeuronCores, tensor-parallel over heads):
  - 2 heads per core; q/k/v/gate projections column-parallel, o_proj
    row-parallel; each core emits a partial [B*S, HID] output which the
    host sums (the all-reduce of row-parallel o_proj).
  - GLA is evaluated in chunked form (chunk C=128):
        Gc   = inclusive cumsum of g within chunk      (PE matmul w/ triangular ones)
        qt   = q * exp(Gc) * scale   (D-major)
        kt   = k * exp(-Gc)          (D-major)
        AT   = kt.T-contract-qt  -> [ck, cq], masked to ck<=cq
        O^T  = S_prev.T-contract-qt + v.T-contract-ATm     (D-major out)
        K2   = kt * exp(Gc_last)  -> transpose -> [i, dk]
        S    = S_prev * exp(Gc_last) + K2.T-contract-v
  - All matmul operand layouts are chosen so no activation transposes are
    needed (host passes x already transposed); only K2 needs a 128x128 PE
    transpose per chunk.

Host-side dispatch:
  - If Wg1==Wg2 and bg1==bg2 the two GLA branches are identical and the
    softmax weights sum to 1, so a single scan suffices.
  - If additionally Wg1 is the identity with zero bias (the distribution
    this problem ships), the gate projection is skipped entirely and the
    gate input is read directly from x columns.
  - Otherwise the (rare) general path runs the single-gate kernel twice
    and combines on the host (o_proj is linear).
"""

import os
import sys

import numpy as np

for _p in ("/opt/trn_rl_repo",):
    if os.path.isdir(_p) and _p not in sys.path:
        sys.path.insert(0, _p)

import concourse.bass as bass
import concourse.mybir as mybir
import concourse.tile as tile
from concourse import bacc
from concourse.bass_utils import run_bass_kernel_spmd

F32 = mybir.dt.float32
AF = mybir.ActivationFunctionType
OP = mybir.AluOpType

B, S, HID = 2, 2048, 2048
H, DH = 16, 128
NCORES = 8
HPC = H // NCORES          # heads per core
DC = HPC * DH              # per-core head dims (256)
TOK = B * S
SLAB = 512
CHUNK = 128
GATE_NORM = 16.0
CLAMP_MIN = -50.0

# Projection-matmul operand dtype: float32 (exact, 4 cy/row) or float32r
# (~1 cy/row at moving dim >= 256, TF32-like precision).  The scan matmuls
# (moving dim 128) gain nothing from f32r and stay exact fp32.
MM_DT = {"f32": F32, "f32r": mybir.dt.float32r}[os.environ.get("GLA_MM_DT", "f32r")]
# Projection/o_proj operand dtype.  fp16 has the same ~11-bit mantissa as
# TF32/f32r but runs 1 cy/row with fast weight load; safe for the bounded
# projection values (|x|<6, |w|<0.2, |attn|<100).  The scan keeps fp32-range
# dtypes because decayed q/k operands span exp(+-44).
PW_DT = {"f32": F32, "f32r": mybir.dt.float32r, "f16": mybir.dt.float16}[
    os.environ.get("GLA_PW_DT", "f16")]
# Scan matmul operand dtype.  bf16 keeps fp32 exponent range (decayed q/k
# operands span exp(+-40), so fp16 would overflow) at 1 cy/row.  The G
# cumsum operands and the recurrent state stay in MM_DT.
SC_DT = {"bf16": mybir.dt.bfloat16, "f32r": mybir.dt.float32r, "f32": F32}[
    os.environ.get("GLA_SC_DT", "f32r")]


def _mm(ap):
    return ap


def build_nc(tok=TOK, gate_mode="identity"):
    """Build the per-core SPMD Bass program.

    gate_mode: "identity" -> gate preactivation is x columns (no projection)
               "general"  -> gate = x @ Wg.T + bg computed on device
    """
    assert tok % SLAB == 0 and (tok // B) % SLAB == 0
    nslabs = tok // SLAB
    slabs_per_b = (tok // B) // SLAB
    n_ct = HID // 128          # contraction tiles
    n_tt = SLAB // CHUNK       # token tiles per slab
    n_eo = HID // 512          # output column tiles

    nc = bacc.Bacc(None, target_bir_lowering=False, debug=False)

    xT = nc.dram_tensor("xT", [HID, tok], PW_DT, kind="ExternalInput")
    wqT = nc.dram_tensor("wqT", [HID, DC], PW_DT, kind="ExternalInput")
    wkT = nc.dram_tensor("wkT", [HID, DC], PW_DT, kind="ExternalInput")
    wvT = nc.dram_tensor("wvT", [HID, DC], PW_DT, kind="ExternalInput")
    woT = nc.dram_tensor("woT", [DC, HID], PW_DT, kind="ExternalInput")
    u1 = nc.dram_tensor("u1", [CHUNK, CHUNK], MM_DT, kind="ExternalInput")
    ident = nc.dram_tensor("ident", [CHUNK, CHUNK], SC_DT, kind="ExternalInput")
    if gate_mode == "identity":
        xcols = nc.dram_tensor("xcols", [tok, DC], F32, kind="ExternalInput")
    else:
        wgT = nc.dram_tensor("wgT", [HID, DC], PW_DT, kind="ExternalInput")
        bg = nc.dram_tensor("bg", [1, DC], PW_DT, kind="ExternalInput")
    out = nc.dram_tensor("out", [tok, HID], F32, kind="ExternalOutput")

    with tile.TileContext(nc) as tc:
        with (
            tc.tile_pool(name="consts", bufs=1) as consts,
            tc.tile_pool(name="xtp", bufs=28) as xtp,
            tc.tile_pool(name="projp", bufs=3) as projp,
            tc.tile_pool(name="tmajp", bufs=8) as tmajp,
            tc.tile_pool(name="tmajt", bufs=5) as tmajt,
            tc.tile_pool(name="scanp", bufs=11) as scanp,
            tc.tile_pool(name="statep", bufs=2) as statep,
            tc.tile_pool(name="attnp", bufs=2) as attnp,
            tc.tile_pool(name="ps_proj", bufs=2, space=bass.MemorySpace.PSUM) as psproj,
            tc.tile_pool(name="ps_scan", bufs=1, space=bass.MemorySpace.PSUM) as psscan,
            tc.tile_pool(name="ps_out", bufs=2, space=bass.MemorySpace.PSUM) as psout,
        ):
            wq_sb = consts.tile([128, n_ct, DC], PW_DT)
            wk_sb = consts.tile([128, n_ct, DC], PW_DT)
            wv_sb = consts.tile([128, n_ct, DC], PW_DT)
            for ct in range(n_ct):
                cs = slice(ct * 128, (ct + 1) * 128)
                nc.sync.dma_start(wq_sb[:, ct, :], wqT[cs, :])
                nc.sync.dma_start(wk_sb[:, ct, :], wkT[cs, :])
            u1_sb = consts.tile([CHUNK, CHUNK], MM_DT)
            nc.sync.dma_start(u1_sb, u1[:, :])
            id_sb = consts.tile([CHUNK, CHUNK], SC_DT)
            nc.sync.dma_start(id_sb, ident[:, :])
            for ct in range(n_ct):
                cs = slice(ct * 128, (ct + 1) * 128)
                nc.sync.dma_start(wv_sb[:, ct, :], wvT[cs, :])
            wo_sb = consts.tile([128, HPC, HID], PW_DT)
            nc.sync.dma_start(wo_sb, woT[:, :].rearrange("(hp p) e -> p hp e", p=128))
            if gate_mode == "general":
                wg_sb = consts.tile([128, n_ct, DC], PW_DT)
                nc.sync.dma_start(wg_sb, wgT[:, :].rearrange("(ct p) d -> p ct d", p=128))
                bg_sb = consts.tile([1, DC], PW_DT)
                nc.sync.dma_start(bg_sb, bg[:, :])
                ones1 = consts.tile([1, CHUNK], PW_DT)
                nc.vector.memset(ones1, 1.0)

            # per-(batch, head) recurrent state [dk, dv]
            s_tiles = {}
            for bh in range(B * HPC):
                t = statep.tile([DH, DH], MM_DT, tag=f"S{bh}")
                nc.vector.memset(t.bitcast(F32), 0.0)
                s_tiles[bh] = t

            for slab in range(nslabs):
                b = slab // slabs_per_b
                t0 = slab * SLAB

                xts = []
                for ct in range(n_ct):
                    xt = xtp.tile([128, SLAB], PW_DT, tag="xt")
                    nc.sync.dma_start(xt, xT[ct * 128:(ct + 1) * 128, t0:t0 + SLAB])
                    xts.append(xt)

                # D-major projections: q, k  (out [d, t])
                qsb, ksb = {}, {}
                for h in range(HPC):
                    for name, wsb, store in (("q", wq_sb, qsb), ("k", wk_sb, ksb)):
                        ps = psproj.tile([128, SLAB], F32, tag="pp")
                        for ct in range(n_ct):
                            nc.tensor.matmul(
                                ps,
                                _mm(wsb[:, ct, h * DH:(h + 1) * DH]),
                                _mm(xts[ct]),
                                start=(ct == 0),
                                stop=(ct == n_ct - 1),
                            )
                        sbt = projp.tile([128, SLAB], F32, tag=name)
                        nc.vector.tensor_copy(sbt, ps)
                        store[h] = sbt

                # T-major projections: v (and gate preact z), out [t, d]
                v_tiles, g_tiles = [], []
                for tt in range(n_tt):
                    ps = psproj.tile([128, DC], F32, tag="pp")
                    for ct in range(n_ct):
                        nc.tensor.matmul(
                            ps,
                            _mm(xts[ct][:, tt * CHUNK:(tt + 1) * CHUNK]),
                            _mm(wv_sb[:, ct, :]),
                            start=(ct == 0),
                            stop=(ct == n_ct - 1),
                        )
                    v_sb = tmajp.tile([128, DC], SC_DT, tag="v")
                    nc.vector.tensor_copy(v_sb, ps)
                    v_tiles.append(v_sb)


                # --- gate pipeline, function-batched (exp x4, ln x4) ---
                sps = []
                for tt in range(n_tt):
                    if gate_mode == "identity":
                        gsrc = tmajt.tile([128, DC], F32, tag="gx")
                        nc.sync.dma_start(
                            gsrc, xcols[t0 + tt * CHUNK:t0 + (tt + 1) * CHUNK, :]
                        )
                        sp = tmajt.tile([128, DC], F32, tag="gsp")
                        nc.scalar.activation(sp, gsrc, AF.Exp, scale=-1.0)
                    else:
                        zps = psproj.tile([128, DC], F32, tag="pp")
                        for ct in range(n_ct):
                            nc.tensor.matmul(
                                zps,
                                _mm(xts[ct][:, tt * CHUNK:(tt + 1) * CHUNK]),
                                _mm(wg_sb[:, ct, :]),
                                start=(ct == 0),
                                stop=False,
                            )
                        nc.tensor.matmul(zps, _mm(ones1), _mm(bg_sb), start=False, stop=True)
                        sp = tmajt.tile([128, DC], F32, tag="gsp")
                        nc.scalar.activation(sp, zps, AF.Exp, scale=-1.0)
                    sps.append(sp)
                for tt in range(n_tt):
                    # log_sigmoid(z) = -ln(1 + exp(-z)); clamped /GATE_NORM
                    lns = tmajt.tile([128, DC], F32, tag="lns")
                    nc.scalar.activation(lns, sps[tt], AF.Ln, bias=1.0)
                    g_sb = tmajp.tile([128, DC], MM_DT, tag="g")
                    nc.vector.tensor_scalar(
                        g_sb, lns, -1.0 / GATE_NORM, CLAMP_MIN, op0=OP.mult, op1=OP.max
                    )
                    g_tiles.append(g_sb)

                # --- batched scan prologue (off the recurrent chain) ---
                # G matmuls + exps + decay muls + AT for all (head, chunk)
                # pairs, emitted function-batched so the ACT engine does not
                # thrash its activation table.
                pre = {}
                for h in range(HPC):
                    for ci in range(n_tt):
                        g_T = g_tiles[ci][:, h * DH:(h + 1) * DH]
                        gd_ps = psscan.tile([DH, CHUNK], F32, tag="gg")
                        nc.tensor.matmul(gd_ps, g_T, u1_sb, start=True, stop=True)
                        expG = scanp.tile([DH, CHUNK], F32, tag="eg")
                        nc.scalar.activation(expG, gd_ps, AF.Exp)
                        expNG = scanp.tile([DH, CHUNK], F32, tag="eng")
                        nc.scalar.activation(expNG, gd_ps, AF.Exp, scale=-1.0)
                        qt = scanp.tile([DH, CHUNK], SC_DT, tag="qt")
                        nc.vector.tensor_mul(
                            qt, qsb[h][:, ci * CHUNK:(ci + 1) * CHUNK], expG
                        )
                        kt = scanp.tile([DH, CHUNK], SC_DT, tag="kt")
                        nc.vector.tensor_mul(
                            kt, ksb[h][:, ci * CHUNK:(ci + 1) * CHUNK], expNG
                        )
                        k2d = scanp.tile([DH, CHUNK], SC_DT, tag="k2d")
                        nc.vector.tensor_scalar_mul(k2d, kt, expG[:, CHUNK - 1:CHUNK])
                        k2t_ps = psscan.tile([CHUNK, DH], SC_DT, tag="kk")
                        nc.tensor.transpose(k2t_ps, k2d, id_sb)
                        k2t = scanp.tile([CHUNK, DH], SC_DT, tag="k2t")
                        nc.scalar.copy(k2t, k2t_ps)
                        at_ps = psscan.tile([CHUNK, CHUNK], F32, tag="ga")
                        nc.tensor.matmul(at_ps, kt, qt, start=True, stop=True)
                        atm = scanp.tile([CHUNK, CHUNK], SC_DT, tag="atm")
                        nc.vector.tensor_mul(atm, at_ps, u1_sb)
                        v_T = v_tiles[ci][:, h * DH:(h + 1) * DH]
                        pre[(h, ci)] = (qt, k2t, atm, v_T, expG)

                # --- recurrent sweep (chunk-serial per head) ---
                attn_tiles = {}
                for h in range(HPC):
                    attn_t = attnp.tile([DH, SLAB], PW_DT, tag=f"at{h}")
                    attn_tiles[h] = attn_t
                for ci in range(n_tt):
                    for h in range(HPC):
                        bh = b * HPC + h
                        qt, k2t, atm, v_T, expG = pre[(h, ci)]
                        s_old = s_tiles[bh]
                        if SC_DT is not MM_DT:
                            s_mm = scanp.tile([DH, DH], SC_DT, tag=f"sm{bh}")
                            nc.vector.tensor_copy(s_mm, s_old)
                        else:
                            s_mm = s_old
                        ot_ps = psscan.tile([DH, CHUNK], F32, tag="ot")
                        nc.tensor.matmul(ot_ps, s_mm, qt, start=True, stop=False)
                        nc.tensor.matmul(ot_ps, v_T, atm, start=False, stop=True)
                        nc.scalar.copy(
                            attn_tiles[h][:, ci * CHUNK:(ci + 1) * CHUNK], ot_ps
                        )
                        kv_ps = psscan.tile([DH, DH], F32, tag="kk")
                        nc.tensor.matmul(kv_ps, k2t, v_T, start=True, stop=True)
                        s_new = statep.tile([DH, DH], MM_DT, tag=f"S{bh}")
                        nc.vector.scalar_tensor_tensor(
                            s_new, s_old, expG[:, CHUNK - 1:CHUNK], kv_ps,
                            op0=OP.mult, op1=OP.add,
                        )
                        s_tiles[bh] = s_new

                # row-parallel o_proj: out[t, e] += attnT[:, t].T @ woT[:, e]
                for tt in range(n_tt):
                    for eo in range(n_eo):
                        ops = psout.tile([CHUNK, 512], F32, tag="o")
                        for h in range(HPC):
                            nc.tensor.matmul(
                                ops,
                                _mm(attn_tiles[h][:, tt * CHUNK:(tt + 1) * CHUNK]),
                                _mm(wo_sb[:, h, eo * 512:(eo + 1) * 512]),
                                start=(h == 0),
                                stop=(h == HPC - 1),
                            )
                        o_sb = projp.tile([CHUNK, 512], F32, tag="ob")
                        nc.vector.tensor_copy(o_sb, ops)
                        nc.gpsimd.dma_start(
                            out[t0 + tt * CHUNK:t0 + (tt + 1) * CHUNK,
                                eo * 512:(eo + 1) * 512],
                            o_sb,
                        )
    nc.compile()
    return nc


_NC_CACHE = {}
LAST_RESULTS = []


def _get_nc(tok, gate_mode):
    key = (tok, gate_mode, MM_DT, PW_DT, SC_DT)
    if key not in _NC_CACHE:
        _NC_CACHE[key] = build_nc(tok, gate_mode)
    return _NC_CACHE[key]


def _make_in_maps(xT, x, Wq, Wk, Wv, Wo, gate_mode, Wg=None, bgv=None, tok=TOK):
    scale = DH ** -0.5
    pw_np = mybir.dt.np(PW_DT)
    u1m = np.triu(np.ones((CHUNK, CHUNK), np.float32))
    idm = np.eye(CHUNK, dtype=np.float32)
    xTp = np.ascontiguousarray(xT.astype(pw_np))
    in_maps = []
    for c in range(NCORES):
        rs = slice(c * DC, (c + 1) * DC)
        m = dict(
            xT=xTp,
            wqT=np.ascontiguousarray((Wq[rs] * scale).T.astype(pw_np)),
            wkT=np.ascontiguousarray(Wk[rs].T.astype(pw_np)),
            wvT=np.ascontiguousarray(Wv[rs].T.astype(pw_np)),
            woT=np.ascontiguousarray(Wo[:, rs].T.astype(pw_np)),
            u1=u1m,
            ident=idm.astype(mybir.dt.np(SC_DT)),
        )
        if gate_mode == "identity":
            m["xcols"] = np.ascontiguousarray(x[:, rs])
        else:
            m["wgT"] = np.ascontiguousarray(Wg[rs].T.astype(pw_np))
            m["bg"] = np.ascontiguousarray(bgv[rs].astype(pw_np)).reshape(1, DC)
        in_maps.append(m)
    return in_maps


def _run(nc, in_maps):
    trace = bool(int(os.environ.get("GLA_TRACE", "0")))
    res = run_bass_kernel_spmd(
        nc, in_maps, list(range(NCORES)), trace=trace,
    )
    LAST_RESULTS.append(res)
    total = res.results[0]["out"].astype(np.float32).copy()
    for i in range(1, NCORES):
        total += res.results[i]["out"]
    return total


def kernel(hidden_states, Wq, Wk, Wv, Wo, Wg1, bg1, Wg2, bg2, alpha_list):
    LAST_RESULTS.clear()
    x = np.ascontiguousarray(np.asarray(hidden_states, np.float32).reshape(TOK, HID))
    xT = np.ascontiguousarray(x.T)
    Wq = np.asarray(Wq, np.float32)
    Wk = np.asarray(Wk, np.float32)
    Wv = np.asarray(Wv, np.float32)
    Wo = np.asarray(Wo, np.float32)
    Wg1 = np.asarray(Wg1, np.float32)
    Wg2 = np.asarray(Wg2, np.float32)
    bg1 = np.asarray(bg1, np.float32)
    bg2 = np.asarray(bg2, np.float32)
    al = np.asarray(alpha_list, np.float64)
    a = np.exp(al - al.max())
    a = (a / a.sum()).astype(np.float32)

    gates_equal = np.array_equal(Wg1, Wg2) and np.array_equal(bg1, bg2)
    ident_gate = (
        gates_equal
        and not bg1.any()
        and np.array_equal(Wg1, np.eye(HID, dtype=np.float32))
    )

    if ident_gate:
        nc = _get_nc(TOK, "identity")
        out = _run(nc, _make_in_maps(xT, x, Wq, Wk, Wv, Wo, "identity"))
    elif gates_equal:
        nc = _get_nc(TOK, "general")
        out = _run(nc, _make_in_maps(xT, x, Wq, Wk, Wv, Wo, "general", Wg1, bg1))
    else:
        nc = _get_nc(TOK, "general")
        o1 = _run(nc, _make_in_maps(xT, x, Wq, Wk, Wv, Wo, "general", Wg1, bg1))
        o2 = _run(nc, _make_in_maps(xT, x, Wq, Wk, Wv, Wo, "general", Wg2, bg2))
        out = a[0] * o1 + a[1] * o2

    return out.reshape(B, S, HID)


# revision 26
# speedup vs baseline: 1.0769x; 1.0769x over previous
"""Trainium2 Bass kernel for nn_DualStateLinearAttention.

Reference math (B=2, S=2048, HID=2048, H=16, D=128):
    q = x @ Wq.T, k = x @ Wk.T, v = x @ Wv.T            (split into 16 heads)
    gk_j = clamp(log_sigmoid(x @ Wgj.T + bgj) / 16, min=-50)   j in {1,2}
    o_j  = GLA scan over S with per-key-dim decay exp(gk_j)
    out  = (softmax(alpha)[0] * o1 + softmax(alpha)[1] * o2) @ Wo.T

Strategy (8 NeuronCores, tensor-parallel over heads):
  - 2 heads per core; q/k/v/gate projections column-parallel, o_proj
    row-parallel; each core emits a partial [B*S, HID] output which the
    host sums (the all-reduce of row-parallel o_proj).
  - GLA is evaluated in chunked form (chunk C=128):
        Gc   = inclusive cumsum of g within chunk      (PE matmul w/ triangular ones)
        qt   = q * exp(Gc) * scale   (D-major)
        kt   = k * exp(-Gc)          (D-major)
        AT   = kt.T-contract-qt  -> [ck, cq], masked to ck<=cq
        O^T  = S_prev.T-contract-qt + v.T-contract-ATm     (D-major out)
        K2   = kt * exp(Gc_last)  -> transpose -> [i, dk]
        S    = S_prev * exp(Gc_last) + K2.T-contract-v
  - All matmul operand layouts are chosen so no activation transposes are
    needed (host passes x already transposed); only K2 needs a 128x128 PE
    transpose per chunk.

Host-side dispatch:
  - If Wg1==Wg2 and bg1==bg2 the two GLA branches are identical and the
    softmax weights sum to 1, so a single scan suffices.
  - If additionally Wg1 is the identity with zero bias (the distribution
    this problem ships), the gate projection is skipped entirely and the
    gate input is read directly from x columns.
  - Otherwise the (rare) general path runs the single-gate kernel twice
    and combines on the host (o_proj is linear).
"""

import os
import sys

import numpy as np

for _p in ("/opt/trn_rl_repo",):
    if os.path.isdir(_p) and _p not in sys.path:
        sys.path.insert(0, _p)

import concourse.bass as bass
import concourse.mybir as mybir
import concourse.tile as tile
from concourse import bacc
from concourse.bass_utils import run_bass_kernel_spmd

F32 = mybir.dt.float32
AF = mybir.ActivationFunctionType
OP = mybir.AluOpType

B, S, HID = 2, 2048, 2048
H, DH = 16, 128
NCORES = 8
HPC = H // NCORES          # heads per core
DC = HPC * DH              # per-core head dims (256)
TOK = B * S
SLAB = 512
CHUNK = 128
GATE_NORM = 16.0
CLAMP_MIN = -50.0

# Projection-matmul operand dtype: float32 (exact, 4 cy/row) or float32r
# (~1 cy/row at moving dim >= 256, TF32-like precision).  The scan matmuls
# (moving dim 128) gain nothing from f32r and stay exact fp32.
MM_DT = {"f32": F32, "f32r": mybir.dt.float32r}[os.environ.get("GLA_MM_DT", "f32r")]
# Projection/o_proj operand dtype.  fp16 has the same ~11-bit mantissa as
# TF32/f32r but runs 1 cy/row with fast weight load; safe for the bounded
# projection values (|x|<6, |w|<0.2, |attn|<100).  The scan keeps fp32-range
# dtypes because decayed q/k operands span exp(+-44).
PW_DT = {"f32": F32, "f32r": mybir.dt.float32r, "f16": mybir.dt.float16}[
    os.environ.get("GLA_PW_DT", "f16")]
# Scan matmul operand dtype.  bf16 keeps fp32 exponent range (decayed q/k
# operands span exp(+-40), so fp16 would overflow) at 1 cy/row.  The G
# cumsum operands and the recurrent state stay in MM_DT.
SC_DT = {"bf16": mybir.dt.bfloat16, "f32r": mybir.dt.float32r, "f32": F32}[
    os.environ.get("GLA_SC_DT", "f32r")]


def _mm(ap):
    return ap


def build_nc(tok=TOK, gate_mode="identity"):
    """Build the per-core SPMD Bass program.

    gate_mode: "identity" -> gate preactivation is x columns (no projection)
               "general"  -> gate = x @ Wg.T + bg computed on device
    """
    assert tok % SLAB == 0 and (tok // B) % SLAB == 0
    nslabs = tok // SLAB
    slabs_per_b = (tok // B) // SLAB
    n_ct = HID // 128          # contraction tiles
    n_tt = SLAB // CHUNK       # token tiles per slab
    n_eo = HID // 512          # output column tiles

    nc = bacc.Bacc(None, target_bir_lowering=False, debug=False)

    xT = nc.dram_tensor("xT", [HID, tok], PW_DT, kind="ExternalInput")
    wqT = nc.dram_tensor("wqT", [HID, DC], PW_DT, kind="ExternalInput")
    wkT = nc.dram_tensor("wkT", [HID, DC], PW_DT, kind="ExternalInput")
    wvT = nc.dram_tensor("wvT", [HID, DC], PW_DT, kind="ExternalInput")
    woT = nc.dram_tensor("woT", [DC, HID], PW_DT, kind="ExternalInput")
    u1 = nc.dram_tensor("u1", [CHUNK, CHUNK], MM_DT, kind="ExternalInput")
    ident = nc.dram_tensor("ident", [CHUNK, CHUNK], SC_DT, kind="ExternalInput")
    if gate_mode == "identity":
        xcols = nc.dram_tensor("xcols", [tok, DC], F32, kind="ExternalInput")
    else:
        wgT = nc.dram_tensor("wgT", [HID, DC], PW_DT, kind="ExternalInput")
        bg = nc.dram_tensor("bg", [1, DC], PW_DT, kind="ExternalInput")
    out = nc.dram_tensor("out", [tok, HID], F32, kind="ExternalOutput")

    with tile.TileContext(nc) as tc:
        with (
            tc.tile_pool(name="consts", bufs=1) as consts,
            tc.tile_pool(name="xtp", bufs=28) as xtp,
            tc.tile_pool(name="projp", bufs=3) as projp,
            tc.tile_pool(name="tmajp", bufs=8) as tmajp,
            tc.tile_pool(name="tmajt", bufs=6) as tmajt,
            tc.tile_pool(name="gatep", bufs=34) as gatep,
            tc.tile_pool(name="scanp", bufs=11) as scanp,
            tc.tile_pool(name="statep", bufs=2) as statep,
            tc.tile_pool(name="attnp", bufs=2) as attnp,
            tc.tile_pool(name="ps_proj", bufs=2, space=bass.MemorySpace.PSUM) as psproj,
            tc.tile_pool(name="ps_scan", bufs=1, space=bass.MemorySpace.PSUM) as psscan,
            tc.tile_pool(name="ps_out", bufs=2, space=bass.MemorySpace.PSUM) as psout,
        ):
            wq_sb = consts.tile([128, n_ct, DC], PW_DT)
            wk_sb = consts.tile([128, n_ct, DC], PW_DT)
            wv_sb = consts.tile([128, n_ct, DC], PW_DT)
            for ct in range(n_ct):
                cs = slice(ct * 128, (ct + 1) * 128)
                nc.sync.dma_start(wq_sb[:, ct, :], wqT[cs, :])
                nc.sync.dma_start(wk_sb[:, ct, :], wkT[cs, :])
            u1_sb = consts.tile([CHUNK, CHUNK], MM_DT)
            nc.sync.dma_start(u1_sb, u1[:, :])
            id_sb = consts.tile([CHUNK, CHUNK], SC_DT)
            nc.sync.dma_start(id_sb, ident[:, :])
            for ct in range(n_ct):
                cs = slice(ct * 128, (ct + 1) * 128)
                nc.sync.dma_start(wv_sb[:, ct, :], wvT[cs, :])
            wo_sb = consts.tile([128, HPC, HID], PW_DT)
            nc.sync.dma_start(wo_sb, woT[:, :].rearrange("(hp p) e -> p hp e", p=128))
            if gate_mode == "general":
                wg_sb = consts.tile([128, n_ct, DC], PW_DT)
                nc.sync.dma_start(wg_sb, wgT[:, :].rearrange("(ct p) d -> p ct d", p=128))
                bg_sb = consts.tile([1, DC], PW_DT)
                nc.sync.dma_start(bg_sb, bg[:, :])
                ones1 = consts.tile([1, CHUNK], PW_DT)
                nc.vector.memset(ones1, 1.0)

            # per-(batch, head) recurrent state [dk, dv]
            s_tiles = {}
            for bh in range(B * HPC):
                t = statep.tile([DH, DH], MM_DT, tag=f"S{bh}")
                nc.vector.memset(t.bitcast(F32), 0.0)
                s_tiles[bh] = t

            # identity-gate mode: all gates depend only on x columns, so
            # compute every chunk's gate up front with exactly one
            # exp-batch and one ln-batch (2 ACT table loads total).
            all_g_tiles = []
            if gate_mode == "identity":
                n_gt = tok // CHUNK
                gsps = []
                for gi in range(n_gt):
                    gsrc = tmajt.tile([128, DC], F32, tag="gx")
                    nc.scalar.dma_start(
                        gsrc, xcols[gi * CHUNK:(gi + 1) * CHUNK, :]
                    )
                    gsp = tmajt.tile([128, DC], F32, tag="gsp")
                    nc.scalar.activation(gsp, gsrc, AF.Exp, scale=-1.0)
                    gsps.append(gsp)
                for gi in range(n_gt):
                    # log_sigmoid(z) = -ln(1 + exp(-z)); clamped /GATE_NORM
                    lns = tmajt.tile([128, DC], F32, tag="lns")
                    nc.scalar.activation(lns, gsps[gi], AF.Ln, bias=1.0)
                    g_sb = gatep.tile([128, DC], MM_DT, tag="g")
                    nc.vector.tensor_scalar(
                        g_sb, lns, -1.0 / GATE_NORM, CLAMP_MIN,
                        op0=OP.mult, op1=OP.max,
                    )
                    all_g_tiles.append(g_sb)

            for slab in range(nslabs):
                b = slab // slabs_per_b
                t0 = slab * SLAB

                xts = []
                for ct in range(n_ct):
                    xt = xtp.tile([128, SLAB], PW_DT, tag="xt")
                    nc.sync.dma_start(xt, xT[ct * 128:(ct + 1) * 128, t0:t0 + SLAB])
                    xts.append(xt)

                # D-major projections: q, k  (out [d, t])
                qsb, ksb = {}, {}
                for h in range(HPC):
                    for name, wsb, store in (("q", wq_sb, qsb), ("k", wk_sb, ksb)):
                        ps = psproj.tile([128, SLAB], F32, tag="pp")
                        for ct in range(n_ct):
                            nc.tensor.matmul(
                                ps,
                                _mm(wsb[:, ct, h * DH:(h + 1) * DH]),
                                _mm(xts[ct]),
                                start=(ct == 0),
                                stop=(ct == n_ct - 1),
                            )
                        sbt = projp.tile([128, SLAB], F32, tag=name)
                        nc.vector.tensor_copy(sbt, ps)
                        store[h] = sbt

                # T-major projections: v (and gate preact z), out [t, d]
                v_tiles, g_tiles = [], []
                for tt in range(n_tt):
                    ps = psproj.tile([128, DC], F32, tag="pp")
                    for ct in range(n_ct):
                        nc.tensor.matmul(
                            ps,
                            _mm(xts[ct][:, tt * CHUNK:(tt + 1) * CHUNK]),
                            _mm(wv_sb[:, ct, :]),
                            start=(ct == 0),
                            stop=(ct == n_ct - 1),
                        )
                    v_sb = tmajp.tile([128, DC], SC_DT, tag="v")
                    nc.vector.tensor_copy(v_sb, ps)
                    v_tiles.append(v_sb)


                if gate_mode == "identity":
                    g_tiles = all_g_tiles[slab * n_tt:(slab + 1) * n_tt]
                else:
                    # gate projection z = x @ Wg.T + bg, then exp/ln batched
                    sps = []
                    for tt in range(n_tt):
                        zps = psproj.tile([128, DC], F32, tag="pp")
                        for ct in range(n_ct):
                            nc.tensor.matmul(
                                zps,
                                _mm(xts[ct][:, tt * CHUNK:(tt + 1) * CHUNK]),
                                _mm(wg_sb[:, ct, :]),
                                start=(ct == 0),
                                stop=False,
                            )
                        nc.tensor.matmul(zps, _mm(ones1), _mm(bg_sb), start=False, stop=True)
                        sp = tmajt.tile([128, DC], F32, tag="gsp")
                        nc.scalar.activation(sp, zps, AF.Exp, scale=-1.0)
                        sps.append(sp)
                    for tt in range(n_tt):
                        lns = tmajt.tile([128, DC], F32, tag="lns")
                        nc.scalar.activation(lns, sps[tt], AF.Ln, bias=1.0)
                        g_sb = tmajp.tile([128, DC], MM_DT, tag="g")
                        nc.vector.tensor_scalar(
                            g_sb, lns, -1.0 / GATE_NORM, CLAMP_MIN,
                            op0=OP.mult, op1=OP.max,
                        )
                        g_tiles.append(g_sb)

                # --- batched scan prologue (off the recurrent chain) ---
                # G matmuls + exps + decay muls + AT for all (head, chunk)
                # pairs, emitted function-batched so the ACT engine does not
                # thrash its activation table.
                pre = {}
                for h in range(HPC):
                    for ci in range(n_tt):
                        g_T = g_tiles[ci][:, h * DH:(h + 1) * DH]
                        gd_ps = psscan.tile([DH, CHUNK], F32, tag="gg")
                        nc.tensor.matmul(gd_ps, g_T, u1_sb, start=True, stop=True)
                        expG = scanp.tile([DH, CHUNK], F32, tag="eg")
                        nc.scalar.activation(expG, gd_ps, AF.Exp)
                        expNG = scanp.tile([DH, CHUNK], F32, tag="eng")
                        nc.scalar.activation(expNG, gd_ps, AF.Exp, scale=-1.0)
                        qt = scanp.tile([DH, CHUNK], SC_DT, tag="qt")
                        nc.vector.tensor_mul(
                            qt, qsb[h][:, ci * CHUNK:(ci + 1) * CHUNK], expG
                        )
                        kt = scanp.tile([DH, CHUNK], SC_DT, tag="kt")
                        nc.vector.tensor_mul(
                            kt, ksb[h][:, ci * CHUNK:(ci + 1) * CHUNK], expNG
                        )
                        k2d = scanp.tile([DH, CHUNK], SC_DT, tag="k2d")
                        nc.vector.tensor_scalar_mul(k2d, kt, expG[:, CHUNK - 1:CHUNK])
                        k2t_ps = psscan.tile([CHUNK, DH], SC_DT, tag="kk")
                        nc.tensor.transpose(k2t_ps, k2d, id_sb)
                        k2t = scanp.tile([CHUNK, DH], SC_DT, tag="k2t")
                        nc.scalar.copy(k2t, k2t_ps)
                        at_ps = psscan.tile([CHUNK, CHUNK], F32, tag="ga")
                        nc.tensor.matmul(at_ps, kt, qt, start=True, stop=True)
                        atm = scanp.tile([CHUNK, CHUNK], SC_DT, tag="atm")
                        nc.vector.tensor_mul(atm, at_ps, u1_sb)
                        v_T = v_tiles[ci][:, h * DH:(h + 1) * DH]
                        pre[(h, ci)] = (qt, k2t, atm, v_T, expG)

                # --- recurrent sweep (chunk-serial per head) ---
                attn_tiles = {}
                for h in range(HPC):
                    attn_t = attnp.tile([DH, SLAB], PW_DT, tag=f"at{h}")
                    attn_tiles[h] = attn_t
                for ci in range(n_tt):
                    for h in range(HPC):
                        bh = b * HPC + h
                        qt, k2t, atm, v_T, expG = pre[(h, ci)]
                        s_old = s_tiles[bh]
                        if SC_DT is not MM_DT:
                            s_mm = scanp.tile([DH, DH], SC_DT, tag=f"sm{bh}")
                            nc.vector.tensor_copy(s_mm, s_old)
                        else:
                            s_mm = s_old
                        ot_ps = psscan.tile([DH, CHUNK], F32, tag="ot")
                        nc.tensor.matmul(ot_ps, s_mm, qt, start=True, stop=False)
                        nc.tensor.matmul(ot_ps, v_T, atm, start=False, stop=True)
                        nc.scalar.copy(
                            attn_tiles[h][:, ci * CHUNK:(ci + 1) * CHUNK], ot_ps
                        )
                        kv_ps = psscan.tile([DH, DH], F32, tag="kk")
                        nc.tensor.matmul(kv_ps, k2t, v_T, start=True, stop=True)
                        s_new = statep.tile([DH, DH], MM_DT, tag=f"S{bh}")
                        nc.vector.scalar_tensor_tensor(
                            s_new, s_old, expG[:, CHUNK - 1:CHUNK], kv_ps,
                            op0=OP.mult, op1=OP.add,
                        )
                        s_tiles[bh] = s_new

                # row-parallel o_proj: out[t, e] += attnT[:, t].T @ woT[:, e]
                for tt in range(n_tt):
                    for eo in range(n_eo):
                        ops = psout.tile([CHUNK, 512], F32, tag="o")
                        for h in range(HPC):
                            nc.tensor.matmul(
                                ops,
                                _mm(attn_tiles[h][:, tt * CHUNK:(tt + 1) * CHUNK]),
                                _mm(wo_sb[:, h, eo * 512:(eo + 1) * 512]),
                                start=(h == 0),
                                stop=(h == HPC - 1),
                            )
                        o_sb = projp.tile([CHUNK, 512], F32, tag="ob")
                        nc.vector.tensor_copy(o_sb, ops)
                        nc.gpsimd.dma_start(
                            out[t0 + tt * CHUNK:t0 + (tt + 1) * CHUNK,
                                eo * 512:(eo + 1) * 512],
                            o_sb,
                        )
    nc.compile()
    return nc


_NC_CACHE = {}
LAST_RESULTS = []


def _get_nc(tok, gate_mode):
    key = (tok, gate_mode, MM_DT, PW_DT, SC_DT)
    if key not in _NC_CACHE:
        _NC_CACHE[key] = build_nc(tok, gate_mode)
    return _NC_CACHE[key]


def _make_in_maps(xT, x, Wq, Wk, Wv, Wo, gate_mode, Wg=None, bgv=None, tok=TOK):
    scale = DH ** -0.5
    pw_np = mybir.dt.np(PW_DT)
    u1m = np.triu(np.ones((CHUNK, CHUNK), np.float32))
    idm = np.eye(CHUNK, dtype=np.float32)
    xTp = np.ascontiguousarray(xT.astype(pw_np))
    in_maps = []
    for c in range(NCORES):
        rs = slice(c * DC, (c + 1) * DC)
        m = dict(
            xT=xTp,
            wqT=np.ascontiguousarray((Wq[rs] * scale).T.astype(pw_np)),
            wkT=np.ascontiguousarray(Wk[rs].T.astype(pw_np)),
            wvT=np.ascontiguousarray(Wv[rs].T.astype(pw_np)),
            woT=np.ascontiguousarray(Wo[:, rs].T.astype(pw_np)),
            u1=u1m,
            ident=idm.astype(mybir.dt.np(SC_DT)),
        )
        if gate_mode == "identity":
            m["xcols"] = np.ascontiguousarray(x[:, rs])
        else:
            m["wgT"] = np.ascontiguousarray(Wg[rs].T.astype(pw_np))
            m["bg"] = np.ascontiguousarray(bgv[rs].astype(pw_np)).reshape(1, DC)
        in_maps.append(m)
    return in_maps


def _run(nc, in_maps):
    trace = bool(int(os.environ.get("GLA_TRACE", "0")))
    res = run_bass_kernel_spmd(
        nc, in_maps, list(range(NCORES)), trace=trace,
    )
    LAST_RESULTS.append(res)
    total = res.results[0]["out"].astype(np.float32).copy()
    for i in range(1, NCORES):
        total += res.results[i]["out"]
    return total


def kernel(hidden_states, Wq, Wk, Wv, Wo, Wg1, bg1, Wg2, bg2, alpha_list):
    LAST_RESULTS.clear()
    x = np.ascontiguousarray(np.asarray(hidden_states, np.float32).reshape(TOK, HID))
    xT = np.ascontiguousarray(x.T)
    Wq = np.asarray(Wq, np.float32)
    Wk = np.asarray(Wk, np.float32)
    Wv = np.asarray(Wv, np.float32)
    Wo = np.asarray(Wo, np.float32)
    Wg1 = np.asarray(Wg1, np.float32)
    Wg2 = np.asarray(Wg2, np.float32)
    bg1 = np.asarray(bg1, np.float32)
    bg2 = np.asarray(bg2, np.float32)
    al = np.asarray(alpha_list, np.float64)
    a = np.exp(al - al.max())
    a = (a / a.sum()).astype(np.float32)

    gates_equal = np.array_equal(Wg1, Wg2) and np.array_equal(bg1, bg2)
    ident_gate = (
        gates_equal
        and not bg1.any()
        and np.array_equal(Wg1, np.eye(HID, dtype=np.float32))
    )

    if ident_gate:
        nc = _get_nc(TOK, "identity")
        out = _run(nc, _make_in_maps(xT, x, Wq, Wk, Wv, Wo, "identity"))
    elif gates_equal:
        nc = _get_nc(TOK, "general")
        out = _run(nc, _make_in_maps(xT, x, Wq, Wk, Wv, Wo, "general", Wg1, bg1))
    else:
        nc = _get_nc(TOK, "general")
        o1 = _run(nc, _make_in_maps(xT, x, Wq, Wk, Wv, Wo, "general", Wg1, bg1))
        o2 = _run(nc, _make_in_maps(xT, x, Wq, Wk, Wv, Wo, "general", Wg2, bg2))
        out = a[0] * o1 + a[1] * o2

    return out.reshape(B, S, HID)


# revision 28
# speedup vs baseline: 1.1429x; 1.0613x over previous
"""Trainium2 Bass kernel for nn_DualStateLinearAttention.

Reference math (B=2, S=2048, HID=2048, H=16, D=128):
    q = x @ Wq.T, k = x @ Wk.T, v = x @ Wv.T            (split into 16 heads)
    gk_j = clamp(log_sigmoid(x @ Wgj.T + bgj) / 16, min=-50)   j in {1,2}
    o_j  = GLA scan over S with per-key-dim decay exp(gk_j)
    out  = (softmax(alpha)[0] * o1 + softmax(alpha)[1] * o2) @ Wo.T

Strategy (8 NeuronCores, tensor-parallel over heads):
  - 2 heads per core; q/k/v/gate projections column-parallel, o_proj
    row-parallel; each core emits a partial [B*S, HID] output which the
    host sums (the all-reduce of row-parallel o_proj).
  - GLA is evaluated in chunked form (chunk C=128):
        Gc   = inclusive cumsum of g within chunk      (PE matmul w/ triangular ones)
        qt   = q * exp(Gc) * scale   (D-major)
        kt   = k * exp(-Gc)          (D-major)
        AT   = kt.T-contract-qt  -> [ck, cq], masked to ck<=cq
        O^T  = S_prev.T-contract-qt + v.T-contract-ATm     (D-major out)
        K2   = kt * exp(Gc_last)  -> transpose -> [i, dk]
        S    = S_prev * exp(Gc_last) + K2.T-contract-v
  - All matmul operand layouts are chosen so no activation transposes are
    needed (host passes x already transposed); only K2 needs a 128x128 PE
    transpose per chunk.

Host-side dispatch:
  - If Wg1==Wg2 and bg1==bg2 the two GLA branches are identical and the
    softmax weights sum to 1, so a single scan suffices.
  - If additionally Wg1 is the identity with zero bias (the distribution
    this problem ships), the gate projection is skipped entirely and the
    gate input is read directly from x columns.
  - Otherwise the (rare) general path runs the single-gate kernel twice
    and combines on the host (o_proj is linear).
"""

import os
import sys

import numpy as np

for _p in ("/opt/trn_rl_repo",):
    if os.path.isdir(_p) and _p not in sys.path:
        sys.path.insert(0, _p)

import concourse.bass as bass
import concourse.mybir as mybir
import concourse.tile as tile
from concourse import bacc
from concourse.bass_utils import run_bass_kernel_spmd

F32 = mybir.dt.float32
AF = mybir.ActivationFunctionType
OP = mybir.AluOpType

B, S, HID = 2, 2048, 2048
H, DH = 16, 128
NCORES = 8
HPC = H // NCORES          # heads per core
DC = HPC * DH              # per-core head dims (256)
TOK = B * S
SLAB = 512
CHUNK = 128
GATE_NORM = 16.0
CLAMP_MIN = -50.0

# Projection-matmul operand dtype: float32 (exact, 4 cy/row) or float32r
# (~1 cy/row at moving dim >= 256, TF32-like precision).  The scan matmuls
# (moving dim 128) gain nothing from f32r and stay exact fp32.
MM_DT = {"f32": F32, "f32r": mybir.dt.float32r}[os.environ.get("GLA_MM_DT", "f32r")]
# Projection/o_proj operand dtype.  fp16 has the same ~11-bit mantissa as
# TF32/f32r but runs 1 cy/row with fast weight load; safe for the bounded
# projection values (|x|<6, |w|<0.2, |attn|<100).  The scan keeps fp32-range
# dtypes because decayed q/k operands span exp(+-44).
PW_DT = {"f32": F32, "f32r": mybir.dt.float32r, "f16": mybir.dt.float16}[
    os.environ.get("GLA_PW_DT", "f16")]
# Scan matmul operand dtype.  bf16 keeps fp32 exponent range (decayed q/k
# operands span exp(+-40), so fp16 would overflow) at 1 cy/row.  The G
# cumsum operands and the recurrent state stay in MM_DT.
SC_DT = {"bf16": mybir.dt.bfloat16, "f32r": mybir.dt.float32r, "f32": F32}[
    os.environ.get("GLA_SC_DT", "f32r")]


def _mm(ap):
    return ap


def build_nc(tok=TOK, gate_mode="identity"):
    """Build the per-core SPMD Bass program.

    gate_mode: "identity" -> gate preactivation is x columns (no projection)
               "general"  -> gate = x @ Wg.T + bg computed on device
    """
    assert tok % SLAB == 0 and (tok // B) % SLAB == 0
    nslabs = tok // SLAB
    slabs_per_b = (tok // B) // SLAB
    n_ct = HID // 128          # contraction tiles
    n_tt = SLAB // CHUNK       # token tiles per slab
    n_eo = HID // 512          # output column tiles

    nc = bacc.Bacc(None, target_bir_lowering=False, debug=False)

    xT = nc.dram_tensor("xT", [HID, tok], PW_DT, kind="ExternalInput")
    wqT = nc.dram_tensor("wqT", [HID, DC], PW_DT, kind="ExternalInput")
    wkT = nc.dram_tensor("wkT", [HID, DC], PW_DT, kind="ExternalInput")
    wvT = nc.dram_tensor("wvT", [HID, DC], PW_DT, kind="ExternalInput")
    woT = nc.dram_tensor("woT", [DC, HID], PW_DT, kind="ExternalInput")
    u1 = nc.dram_tensor("u1", [CHUNK, CHUNK], MM_DT, kind="ExternalInput")
    ident = nc.dram_tensor("ident", [CHUNK, CHUNK], SC_DT, kind="ExternalInput")
    if gate_mode == "identity":
        xcols = nc.dram_tensor("xcols", [tok, DC], PW_DT, kind="ExternalInput")
    else:
        wgT = nc.dram_tensor("wgT", [HID, DC], PW_DT, kind="ExternalInput")
        bg = nc.dram_tensor("bg", [1, DC], PW_DT, kind="ExternalInput")
    out = nc.dram_tensor("out", [tok, HID], F32, kind="ExternalOutput")

    with tile.TileContext(nc) as tc:
        with (
            tc.tile_pool(name="consts", bufs=1) as consts,
            tc.tile_pool(name="xtp", bufs=26) as xtp,
            tc.tile_pool(name="projp", bufs=3) as projp,
            tc.tile_pool(name="tmajp", bufs=8) as tmajp,
            tc.tile_pool(name="tmajt", bufs=10) as tmajt,
            tc.tile_pool(name="gatep", bufs=34) as gatep,
            tc.tile_pool(name="scanp", bufs=10) as scanp,
            tc.tile_pool(name="statep", bufs=2) as statep,
            tc.tile_pool(name="attnp", bufs=2) as attnp,
            tc.tile_pool(name="ps_proj", bufs=2, space=bass.MemorySpace.PSUM) as psproj,
            tc.tile_pool(name="ps_scan", bufs=1, space=bass.MemorySpace.PSUM) as psscan,
            tc.tile_pool(name="ps_out", bufs=2, space=bass.MemorySpace.PSUM) as psout,
        ):
            wq_sb = consts.tile([128, n_ct, DC], PW_DT)
            wk_sb = consts.tile([128, n_ct, DC], PW_DT)
            wv_sb = consts.tile([128, n_ct, DC], PW_DT)
            for ct in range(n_ct):
                cs = slice(ct * 128, (ct + 1) * 128)
                nc.sync.dma_start(wq_sb[:, ct, :], wqT[cs, :])
                nc.sync.dma_start(wk_sb[:, ct, :], wkT[cs, :])
            u1_sb = consts.tile([CHUNK, CHUNK], MM_DT)
            nc.sync.dma_start(u1_sb, u1[:, :])
            id_sb = consts.tile([CHUNK, CHUNK], SC_DT)
            nc.sync.dma_start(id_sb, ident[:, :])
            wo_sb = consts.tile([128, HPC, HID], PW_DT)
            if gate_mode == "general":
                wg_sb = consts.tile([128, n_ct, DC], PW_DT)
                nc.sync.dma_start(wg_sb, wgT[:, :].rearrange("(ct p) d -> p ct d", p=128))
                bg_sb = consts.tile([1, DC], PW_DT)
                nc.sync.dma_start(bg_sb, bg[:, :])
                ones1 = consts.tile([1, CHUNK], PW_DT)
                nc.vector.memset(ones1, 1.0)

            # per-(batch, head) recurrent state [dk, dv]
            s_tiles = {}
            for bh in range(B * HPC):
                t = statep.tile([DH, DH], MM_DT, tag=f"S{bh}")
                nc.vector.memset(t.bitcast(F32), 0.0)
                s_tiles[bh] = t

            # identity-gate mode: all gates depend only on x columns, so
            # compute every chunk's gate up front with exactly one
            # exp-batch and one ln-batch (2 ACT table loads total).
            all_g_tiles = []
            if gate_mode == "identity":
                n_gt = tok // CHUNK
                for g0 in range(0, n_gt, 8):
                    gsps = []
                    for gi in range(g0, g0 + 8):
                        gsrc = tmajt.tile([128, DC], PW_DT, tag="gx")
                        nc.scalar.dma_start(
                            gsrc, xcols[gi * CHUNK:(gi + 1) * CHUNK, :]
                        )
                        gsp = tmajt.tile([128, DC], F32, tag="gsp")
                        nc.scalar.activation(gsp, gsrc, AF.Exp, scale=-1.0)
                        gsps.append(gsp)
                    for gj, gi in enumerate(range(g0, g0 + 8)):
                        # log_sigmoid(z) = -ln(1 + exp(-z)); clamped /GATE_NORM
                        lns = tmajt.tile([128, DC], F32, tag="lns")
                        nc.scalar.activation(lns, gsps[gj], AF.Ln, bias=1.0)
                        g_sb = gatep.tile([128, DC], MM_DT, tag="g")
                        nc.vector.tensor_scalar(
                            g_sb, lns, -1.0 / GATE_NORM, CLAMP_MIN,
                            op0=OP.mult, op1=OP.max,
                        )
                        all_g_tiles.append(g_sb)

            for slab in range(nslabs):
                b = slab // slabs_per_b
                t0 = slab * SLAB

                xts = []
                for ct in range(n_ct):
                    xt = xtp.tile([128, SLAB], PW_DT, tag="xt")
                    nc.sync.dma_start(xt, xT[ct * 128:(ct + 1) * 128, t0:t0 + SLAB])
                    xts.append(xt)
                if slab == 0:
                    for ct in range(n_ct):
                        cs = slice(ct * 128, (ct + 1) * 128)
                        nc.sync.dma_start(wv_sb[:, ct, :], wvT[cs, :])
                    nc.sync.dma_start(
                        wo_sb, woT[:, :].rearrange("(hp p) e -> p hp e", p=128)
                    )

                # D-major projections: q, k  (out [d, t])
                qsb, ksb = {}, {}
                for h in range(HPC):
                    for name, wsb, store in (("q", wq_sb, qsb), ("k", wk_sb, ksb)):
                        ps = psproj.tile([128, SLAB], F32, tag="pp")
                        for ct in range(n_ct):
                            nc.tensor.matmul(
                                ps,
                                _mm(wsb[:, ct, h * DH:(h + 1) * DH]),
                                _mm(xts[ct]),
                                start=(ct == 0),
                                stop=(ct == n_ct - 1),
                            )
                        sbt = projp.tile([128, SLAB], F32, tag=name)
                        nc.vector.tensor_copy(sbt, ps)
                        store[h] = sbt

                # T-major projections: v (and gate preact z), out [t, d]
                v_tiles, g_tiles = [], []
                for tt in range(n_tt):
                    ps = psproj.tile([128, DC], F32, tag="pp")
                    for ct in range(n_ct):
                        nc.tensor.matmul(
                            ps,
                            _mm(xts[ct][:, tt * CHUNK:(tt + 1) * CHUNK]),
                            _mm(wv_sb[:, ct, :]),
                            start=(ct == 0),
                            stop=(ct == n_ct - 1),
                        )
                    v_sb = tmajp.tile([128, DC], SC_DT, tag="v")
                    nc.vector.tensor_copy(v_sb, ps)
                    v_tiles.append(v_sb)


                if gate_mode == "identity":
                    g_tiles = all_g_tiles[slab * n_tt:(slab + 1) * n_tt]
                else:
                    # gate projection z = x @ Wg.T + bg, then exp/ln batched
                    sps = []
                    for tt in range(n_tt):
                        zps = psproj.tile([128, DC], F32, tag="pp")
                        for ct in range(n_ct):
                            nc.tensor.matmul(
                                zps,
                                _mm(xts[ct][:, tt * CHUNK:(tt + 1) * CHUNK]),
                                _mm(wg_sb[:, ct, :]),
                                start=(ct == 0),
                                stop=False,
                            )
                        nc.tensor.matmul(zps, _mm(ones1), _mm(bg_sb), start=False, stop=True)
                        sp = tmajt.tile([128, DC], F32, tag="gsp")
                        nc.scalar.activation(sp, zps, AF.Exp, scale=-1.0)
                        sps.append(sp)
                    for tt in range(n_tt):
                        lns = tmajt.tile([128, DC], F32, tag="lns")
                        nc.scalar.activation(lns, sps[tt], AF.Ln, bias=1.0)
                        g_sb = tmajp.tile([128, DC], MM_DT, tag="g")
                        nc.vector.tensor_scalar(
                            g_sb, lns, -1.0 / GATE_NORM, CLAMP_MIN,
                            op0=OP.mult, op1=OP.max,
                        )
                        g_tiles.append(g_sb)

                # --- batched scan prologue (off the recurrent chain) ---
                # G matmuls + exps + decay muls + AT for all (head, chunk)
                # pairs, emitted function-batched so the ACT engine does not
                # thrash its activation table.
                pre = {}
                for h in range(HPC):
                    for ci in range(n_tt):
                        g_T = g_tiles[ci][:, h * DH:(h + 1) * DH]
                        gd_ps = psscan.tile([DH, CHUNK], F32, tag="gg")
                        nc.tensor.matmul(gd_ps, g_T, u1_sb, start=True, stop=True)
                        expG = scanp.tile([DH, CHUNK], F32, tag="eg")
                        nc.scalar.activation(expG, gd_ps, AF.Exp)
                        expNG = scanp.tile([DH, CHUNK], F32, tag="eng")
                        nc.scalar.activation(expNG, gd_ps, AF.Exp, scale=-1.0)
                        qt = scanp.tile([DH, CHUNK], SC_DT, tag="qt")
                        nc.vector.tensor_mul(
                            qt, qsb[h][:, ci * CHUNK:(ci + 1) * CHUNK], expG
                        )
                        kt = scanp.tile([DH, CHUNK], SC_DT, tag="kt")
                        nc.vector.tensor_mul(
                            kt, ksb[h][:, ci * CHUNK:(ci + 1) * CHUNK], expNG
                        )
                        k2d = scanp.tile([DH, CHUNK], SC_DT, tag="k2d")
                        nc.vector.tensor_scalar_mul(k2d, kt, expG[:, CHUNK - 1:CHUNK])
                        k2t_ps = psscan.tile([CHUNK, DH], SC_DT, tag="kk")
                        nc.tensor.transpose(k2t_ps, k2d, id_sb)
                        k2t = scanp.tile([CHUNK, DH], SC_DT, tag="k2t")
                        nc.scalar.copy(k2t, k2t_ps)
                        at_ps = psscan.tile([CHUNK, CHUNK], F32, tag="ga")
                        nc.tensor.matmul(at_ps, kt, qt, start=True, stop=True)
                        atm = scanp.tile([CHUNK, CHUNK], SC_DT, tag="atm")
                        nc.vector.tensor_mul(atm, at_ps, u1_sb)
                        v_T = v_tiles[ci][:, h * DH:(h + 1) * DH]
                        pre[(h, ci)] = (qt, k2t, atm, v_T, expG)

                # --- recurrent sweep (chunk-serial per head) ---
                attn_tiles = {}
                for h in range(HPC):
                    attn_t = attnp.tile([DH, SLAB], PW_DT, tag=f"at{h}")
                    attn_tiles[h] = attn_t
                for ci in range(n_tt):
                    for h in range(HPC):
                        bh = b * HPC + h
                        qt, k2t, atm, v_T, expG = pre[(h, ci)]
                        s_old = s_tiles[bh]
                        if SC_DT is not MM_DT:
                            s_mm = scanp.tile([DH, DH], SC_DT, tag=f"sm{bh}")
                            nc.vector.tensor_copy(s_mm, s_old)
                        else:
                            s_mm = s_old
                        ot_ps = psscan.tile([DH, CHUNK], F32, tag="ot")
                        nc.tensor.matmul(ot_ps, s_mm, qt, start=True, stop=False)
                        nc.tensor.matmul(ot_ps, v_T, atm, start=False, stop=True)
                        nc.scalar.copy(
                            attn_tiles[h][:, ci * CHUNK:(ci + 1) * CHUNK], ot_ps
                        )
                        kv_ps = psscan.tile([DH, DH], F32, tag="kk")
                        nc.tensor.matmul(kv_ps, k2t, v_T, start=True, stop=True)
                        s_new = statep.tile([DH, DH], MM_DT, tag=f"S{bh}")
                        nc.vector.scalar_tensor_tensor(
                            s_new, s_old, expG[:, CHUNK - 1:CHUNK], kv_ps,
                            op0=OP.mult, op1=OP.add,
                        )
                        s_tiles[bh] = s_new

                # row-parallel o_proj: out[t, e] += attnT[:, t].T @ woT[:, e]
                for tt in range(n_tt):
                    for eo in range(n_eo):
                        ops = psout.tile([CHUNK, 512], F32, tag="o")
                        for h in range(HPC):
                            nc.tensor.matmul(
                                ops,
                                _mm(attn_tiles[h][:, tt * CHUNK:(tt + 1) * CHUNK]),
                                _mm(wo_sb[:, h, eo * 512:(eo + 1) * 512]),
                                start=(h == 0),
                                stop=(h == HPC - 1),
                            )
                        o_sb = projp.tile([CHUNK, 512], F32, tag="ob")
                        nc.vector.tensor_copy(o_sb, ops)
                        nc.gpsimd.dma_start(
                            out[t0 + tt * CHUNK:t0 + (tt + 1) * CHUNK,
                                eo * 512:(eo + 1) * 512],
                            o_sb,
                        )
    nc.compile()
    return nc


_NC_CACHE = {}
LAST_RESULTS = []


def _get_nc(tok, gate_mode):
    key = (tok, gate_mode, MM_DT, PW_DT, SC_DT)
    if key not in _NC_CACHE:
        _NC_CACHE[key] = build_nc(tok, gate_mode)
    return _NC_CACHE[key]


def _make_in_maps(xT, x, Wq, Wk, Wv, Wo, gate_mode, Wg=None, bgv=None, tok=TOK):
    scale = DH ** -0.5
    pw_np = mybir.dt.np(PW_DT)
    u1m = np.triu(np.ones((CHUNK, CHUNK), np.float32))
    idm = np.eye(CHUNK, dtype=np.float32)
    xTp = np.ascontiguousarray(xT.astype(pw_np))
    in_maps = []
    for c in range(NCORES):
        rs = slice(c * DC, (c + 1) * DC)
        m = dict(
            xT=xTp,
            wqT=np.ascontiguousarray((Wq[rs] * scale).T.astype(pw_np)),
            wkT=np.ascontiguousarray(Wk[rs].T.astype(pw_np)),
            wvT=np.ascontiguousarray(Wv[rs].T.astype(pw_np)),
            woT=np.ascontiguousarray(Wo[:, rs].T.astype(pw_np)),
            u1=u1m,
            ident=idm.astype(mybir.dt.np(SC_DT)),
        )
        if gate_mode == "identity":
            m["xcols"] = np.ascontiguousarray(x[:, rs].astype(pw_np))
        else:
            m["wgT"] = np.ascontiguousarray(Wg[rs].T.astype(pw_np))
            m["bg"] = np.ascontiguousarray(bgv[rs].astype(pw_np)).reshape(1, DC)
        in_maps.append(m)
    return in_maps


def _run(nc, in_maps):
    trace = bool(int(os.environ.get("GLA_TRACE", "0")))
    res = run_bass_kernel_spmd(
        nc, in_maps, list(range(NCORES)), trace=trace,
    )
    LAST_RESULTS.append(res)
    total = res.results[0]["out"].astype(np.float32).copy()
    for i in range(1, NCORES):
        total += res.results[i]["out"]
    return total


def kernel(hidden_states, Wq, Wk, Wv, Wo, Wg1, bg1, Wg2, bg2, alpha_list):
    LAST_RESULTS.clear()
    x = np.ascontiguousarray(np.asarray(hidden_states, np.float32).reshape(TOK, HID))
    xT = np.ascontiguousarray(x.T)
    Wq = np.asarray(Wq, np.float32)
    Wk = np.asarray(Wk, np.float32)
    Wv = np.asarray(Wv, np.float32)
    Wo = np.asarray(Wo, np.float32)
    Wg1 = np.asarray(Wg1, np.float32)
    Wg2 = np.asarray(Wg2, np.float32)
    bg1 = np.asarray(bg1, np.float32)
    bg2 = np.asarray(bg2, np.float32)
    al = np.asarray(alpha_list, np.float64)
    a = np.exp(al - al.max())
    a = (a / a.sum()).astype(np.float32)

    gates_equal = np.array_equal(Wg1, Wg2) and np.array_equal(bg1, bg2)
    ident_gate = (
        gates_equal
        and not bg1.any()
        and np.array_equal(Wg1, np.eye(HID, dtype=np.float32))
    )

    if ident_gate:
        nc = _get_nc(TOK, "identity")
        out = _run(nc, _make_in_maps(xT, x, Wq, Wk, Wv, Wo, "identity"))
    elif gates_equal:
        nc = _get_nc(TOK, "general")
        out = _run(nc, _make_in_maps(xT, x, Wq, Wk, Wv, Wo, "general", Wg1, bg1))
    else:
        nc = _get_nc(TOK, "general")
        o1 = _run(nc, _make_in_maps(xT, x, Wq, Wk, Wv, Wo, "general", Wg1, bg1))
        o2 = _run(nc, _make_in_maps(xT, x, Wq, Wk, Wv, Wo, "general", Wg2, bg2))
        out = a[0] * o1 + a[1] * o2

    return out.reshape(B, S, HID)


# revision 30
# speedup vs baseline: 1.1862x; 1.0379x over previous
"""Trainium2 Bass kernel for nn_DualStateLinearAttention.

Reference math (B=2, S=2048, HID=2048, H=16, D=128):
    q = x @ Wq.T, k = x @ Wk.T, v = x @ Wv.T            (split into 16 heads)
    gk_j = clamp(log_sigmoid(x @ Wgj.T + bgj) / 16, min=-50)   j in {1,2}
    o_j  = GLA scan over S with per-key-dim decay exp(gk_j)
    out  = (softmax(alpha)[0] * o1 + softmax(alpha)[1] * o2) @ Wo.T

Strategy (8 NeuronCores, tensor-parallel over heads):
  - 2 heads per core; q/k/v/gate projections column-parallel, o_proj
    row-parallel; each core emits a partial [B*S, HID] output which the
    host sums (the all-reduce of row-parallel o_proj).
  - GLA is evaluated in chunked form (chunk C=128):
        Gc   = inclusive cumsum of g within chunk      (PE matmul w/ triangular ones)
        qt   = q * exp(Gc) * scale   (D-major)
        kt   = k * exp(-Gc)          (D-major)
        AT   = kt.T-contract-qt  -> [ck, cq], masked to ck<=cq
        O^T  = S_prev.T-contract-qt + v.T-contract-ATm     (D-major out)
        K2   = kt * exp(Gc_last)  -> transpose -> [i, dk]
        S    = S_prev * exp(Gc_last) + K2.T-contract-v
  - All matmul operand layouts are chosen so no activation transposes are
    needed (host passes x already transposed); only K2 needs a 128x128 PE
    transpose per chunk.

Host-side dispatch:
  - If Wg1==Wg2 and bg1==bg2 the two GLA branches are identical and the
    softmax weights sum to 1, so a single scan suffices.
  - If additionally Wg1 is the identity with zero bias (the distribution
    this problem ships), the gate projection is skipped entirely and the
    gate input is read directly from x columns.
  - Otherwise the (rare) general path runs the single-gate kernel twice
    and combines on the host (o_proj is linear).
"""

import os
import sys

import numpy as np

for _p in ("/opt/trn_rl_repo",):
    if os.path.isdir(_p) and _p not in sys.path:
        sys.path.insert(0, _p)

import concourse.bass as bass
import concourse.mybir as mybir
import concourse.tile as tile
from concourse import bacc
from concourse.bass_utils import run_bass_kernel_spmd

F32 = mybir.dt.float32
AF = mybir.ActivationFunctionType
OP = mybir.AluOpType

B, S, HID = 2, 2048, 2048
H, DH = 16, 128
NCORES = 8
HPC = H // NCORES          # heads per core
DC = HPC * DH              # per-core head dims (256)
TOK = B * S
SLAB = 512
CHUNK = 128
GATE_NORM = 16.0
CLAMP_MIN = -50.0

# Projection-matmul operand dtype: float32 (exact, 4 cy/row) or float32r
# (~1 cy/row at moving dim >= 256, TF32-like precision).  The scan matmuls
# (moving dim 128) gain nothing from f32r and stay exact fp32.
MM_DT = {"f32": F32, "f32r": mybir.dt.float32r}[os.environ.get("GLA_MM_DT", "f32r")]
# Projection/o_proj operand dtype.  fp16 has the same ~11-bit mantissa as
# TF32/f32r but runs 1 cy/row with fast weight load; safe for the bounded
# projection values (|x|<6, |w|<0.2, |attn|<100).  The scan keeps fp32-range
# dtypes because decayed q/k operands span exp(+-44).
PW_DT = {"f32": F32, "f32r": mybir.dt.float32r, "f16": mybir.dt.float16}[
    os.environ.get("GLA_PW_DT", "f16")]
# Scan matmul operand dtype.  bf16 keeps fp32 exponent range (decayed q/k
# operands span exp(+-40), so fp16 would overflow) at 1 cy/row.  The G
# cumsum operands and the recurrent state stay in MM_DT.
SC_DT = {"bf16": mybir.dt.bfloat16, "f32r": mybir.dt.float32r, "f32": F32}[
    os.environ.get("GLA_SC_DT", "f32r")]


def _mm(ap):
    return ap


def build_nc(tok=TOK, gate_mode="identity"):
    """Build the per-core SPMD Bass program.

    gate_mode: "identity" -> gate preactivation is x columns (no projection)
               "general"  -> gate = x @ Wg.T + bg computed on device
    """
    assert tok % SLAB == 0 and (tok // B) % SLAB == 0
    nslabs = tok // SLAB
    slabs_per_b = (tok // B) // SLAB
    n_ct = HID // 128          # contraction tiles
    n_tt = SLAB // CHUNK       # token tiles per slab
    n_eo = HID // 512          # output column tiles

    nc = bacc.Bacc(None, target_bir_lowering=False, debug=False)

    xT = nc.dram_tensor("xT", [HID, tok], PW_DT, kind="ExternalInput")
    wqT = nc.dram_tensor("wqT", [HID, DC], PW_DT, kind="ExternalInput")
    wkT = nc.dram_tensor("wkT", [HID, DC], PW_DT, kind="ExternalInput")
    wvT = nc.dram_tensor("wvT", [HID, DC], PW_DT, kind="ExternalInput")
    woT = nc.dram_tensor("woT", [DC, HID], PW_DT, kind="ExternalInput")
    u1 = nc.dram_tensor("u1", [CHUNK, CHUNK], MM_DT, kind="ExternalInput")
    ident = nc.dram_tensor("ident", [CHUNK, CHUNK], SC_DT, kind="ExternalInput")
    if gate_mode == "identity":
        xcols = nc.dram_tensor("xcols", [tok, DC], PW_DT, kind="ExternalInput")
    else:
        wgT = nc.dram_tensor("wgT", [HID, DC], PW_DT, kind="ExternalInput")
        bg = nc.dram_tensor("bg", [1, DC], PW_DT, kind="ExternalInput")
    out = nc.dram_tensor("out", [tok, HID], F32, kind="ExternalOutput")

    with tile.TileContext(nc) as tc:
        with (
            tc.tile_pool(name="consts", bufs=1) as consts,
            tc.tile_pool(name="xtp", bufs=26) as xtp,
            tc.tile_pool(name="projp", bufs=3) as projp,
            tc.tile_pool(name="tmajp", bufs=8) as tmajp,
            tc.tile_pool(name="tmajt", bufs=10) as tmajt,
            tc.tile_pool(name="gatep", bufs=34) as gatep,
            tc.tile_pool(name="scanp", bufs=10) as scanp,
            tc.tile_pool(name="statep", bufs=2) as statep,
            tc.tile_pool(name="attnp", bufs=2) as attnp,
            tc.tile_pool(name="ps_proj", bufs=2, space=bass.MemorySpace.PSUM) as psproj,
            tc.tile_pool(name="ps_scan", bufs=1, space=bass.MemorySpace.PSUM) as psscan,
            tc.tile_pool(name="ps_out", bufs=2, space=bass.MemorySpace.PSUM) as psout,
        ):
            wq_sb = consts.tile([128, n_ct, DC], PW_DT)
            wk_sb = consts.tile([128, n_ct, DC], PW_DT)
            wv_sb = consts.tile([128, n_ct, DC], PW_DT)
            wo_sb = consts.tile([128, HPC, HID], PW_DT)
            for ct in range(n_ct):
                cs = slice(ct * 128, (ct + 1) * 128)
                nc.scalar.dma_start(wq_sb[:, ct, :], wqT[cs, :])
                nc.scalar.dma_start(wk_sb[:, ct, :], wkT[cs, :])
            u1_sb = consts.tile([CHUNK, CHUNK], MM_DT)
            nc.scalar.dma_start(u1_sb, u1[:, :])
            id_sb = consts.tile([CHUNK, CHUNK], SC_DT)
            nc.scalar.dma_start(id_sb, ident[:, :])
            for ct in range(n_ct):
                cs = slice(ct * 128, (ct + 1) * 128)
                nc.gpsimd.dma_start(wv_sb[:, ct, :], wvT[cs, :])
            nc.gpsimd.dma_start(
                wo_sb, woT[:, :].rearrange("(hp p) e -> p hp e", p=128)
            )
            if gate_mode == "general":
                wg_sb = consts.tile([128, n_ct, DC], PW_DT)
                nc.sync.dma_start(wg_sb, wgT[:, :].rearrange("(ct p) d -> p ct d", p=128))
                bg_sb = consts.tile([1, DC], PW_DT)
                nc.sync.dma_start(bg_sb, bg[:, :])
                ones1 = consts.tile([1, CHUNK], PW_DT)
                nc.vector.memset(ones1, 1.0)

            # per-(batch, head) recurrent state [dk, dv]
            s_tiles = {}
            for bh in range(B * HPC):
                t = statep.tile([DH, DH], MM_DT, tag=f"S{bh}")
                nc.vector.memset(t.bitcast(F32), 0.0)
                s_tiles[bh] = t

            # identity-gate mode: all gates depend only on x columns, so
            # compute every chunk's gate up front with exactly one
            # exp-batch and one ln-batch (2 ACT table loads total).
            xts0 = []
            for ct in range(n_ct):
                xt = xtp.tile([128, SLAB], PW_DT, tag="xt")
                nc.sync.dma_start(xt, xT[ct * 128:(ct + 1) * 128, 0:SLAB])
                xts0.append(xt)

            all_g_tiles = []
            if gate_mode == "identity":
                n_gt = tok // CHUNK
                for g0 in range(0, n_gt, 8):
                    gsps = []
                    for gi in range(g0, g0 + 8):
                        gsrc = tmajt.tile([128, DC], PW_DT, tag="gx")
                        nc.sync.dma_start(
                            gsrc, xcols[gi * CHUNK:(gi + 1) * CHUNK, :]
                        )
                        gsp = tmajt.tile([128, DC], F32, tag="gsp")
                        nc.scalar.activation(gsp, gsrc, AF.Exp, scale=-1.0)
                        gsps.append(gsp)
                    for gj, gi in enumerate(range(g0, g0 + 8)):
                        # log_sigmoid(z) = -ln(1 + exp(-z)); clamped /GATE_NORM
                        lns = tmajt.tile([128, DC], F32, tag="lns")
                        nc.scalar.activation(lns, gsps[gj], AF.Ln, bias=1.0)
                        g_sb = gatep.tile([128, DC], MM_DT, tag="g")
                        nc.vector.tensor_scalar(
                            g_sb, lns, -1.0 / GATE_NORM, CLAMP_MIN,
                            op0=OP.mult, op1=OP.max,
                        )
                        all_g_tiles.append(g_sb)

            for slab in range(nslabs):
                b = slab // slabs_per_b
                t0 = slab * SLAB

                if slab == 0:
                    xts = xts0
                else:
                    xts = []
                    for ct in range(n_ct):
                        xt = xtp.tile([128, SLAB], PW_DT, tag="xt")
                        nc.sync.dma_start(
                            xt, xT[ct * 128:(ct + 1) * 128, t0:t0 + SLAB]
                        )
                        xts.append(xt)

                # D-major projections: q, k  (out [d, t])
                qsb, ksb = {}, {}
                for h in range(HPC):
                    for name, wsb, store in (("q", wq_sb, qsb), ("k", wk_sb, ksb)):
                        ps = psproj.tile([128, SLAB], F32, tag="pp")
                        for ct in range(n_ct):
                            nc.tensor.matmul(
                                ps,
                                _mm(wsb[:, ct, h * DH:(h + 1) * DH]),
                                _mm(xts[ct]),
                                start=(ct == 0),
                                stop=(ct == n_ct - 1),
                            )
                        sbt = projp.tile([128, SLAB], F32, tag=name)
                        nc.vector.tensor_copy(sbt, ps)
                        store[h] = sbt

                # T-major projections: v (and gate preact z), out [t, d]
                v_tiles, g_tiles = [], []
                for tt in range(n_tt):
                    ps = psproj.tile([128, DC], F32, tag="pp")
                    for ct in range(n_ct):
                        nc.tensor.matmul(
                            ps,
                            _mm(xts[ct][:, tt * CHUNK:(tt + 1) * CHUNK]),
                            _mm(wv_sb[:, ct, :]),
                            start=(ct == 0),
                            stop=(ct == n_ct - 1),
                        )
                    v_sb = tmajp.tile([128, DC], SC_DT, tag="v")
                    nc.vector.tensor_copy(v_sb, ps)
                    v_tiles.append(v_sb)


                if gate_mode == "identity":
                    g_tiles = all_g_tiles[slab * n_tt:(slab + 1) * n_tt]
                else:
                    # gate projection z = x @ Wg.T + bg, then exp/ln batched
                    sps = []
                    for tt in range(n_tt):
                        zps = psproj.tile([128, DC], F32, tag="pp")
                        for ct in range(n_ct):
                            nc.tensor.matmul(
                                zps,
                                _mm(xts[ct][:, tt * CHUNK:(tt + 1) * CHUNK]),
                                _mm(wg_sb[:, ct, :]),
                                start=(ct == 0),
                                stop=False,
                            )
                        nc.tensor.matmul(zps, _mm(ones1), _mm(bg_sb), start=False, stop=True)
                        sp = tmajt.tile([128, DC], F32, tag="gsp")
                        nc.scalar.activation(sp, zps, AF.Exp, scale=-1.0)
                        sps.append(sp)
                    for tt in range(n_tt):
                        lns = tmajt.tile([128, DC], F32, tag="lns")
                        nc.scalar.activation(lns, sps[tt], AF.Ln, bias=1.0)
                        g_sb = tmajp.tile([128, DC], MM_DT, tag="g")
                        nc.vector.tensor_scalar(
                            g_sb, lns, -1.0 / GATE_NORM, CLAMP_MIN,
                            op0=OP.mult, op1=OP.max,
                        )
                        g_tiles.append(g_sb)

                # --- batched scan prologue (off the recurrent chain) ---
                # G matmuls + exps + decay muls + AT for all (head, chunk)
                # pairs, emitted function-batched so the ACT engine does not
                # thrash its activation table.
                pre = {}
                for h in range(HPC):
                    for ci in range(n_tt):
                        g_T = g_tiles[ci][:, h * DH:(h + 1) * DH]
                        gd_ps = psscan.tile([DH, CHUNK], F32, tag="gg")
                        nc.tensor.matmul(gd_ps, g_T, u1_sb, start=True, stop=True)
                        expG = scanp.tile([DH, CHUNK], F32, tag="eg")
                        nc.scalar.activation(expG, gd_ps, AF.Exp)
                        expNG = scanp.tile([DH, CHUNK], F32, tag="eng")
                        nc.scalar.activation(expNG, gd_ps, AF.Exp, scale=-1.0)
                        qt = scanp.tile([DH, CHUNK], SC_DT, tag="qt")
                        nc.vector.tensor_mul(
                            qt, qsb[h][:, ci * CHUNK:(ci + 1) * CHUNK], expG
                        )
                        kt = scanp.tile([DH, CHUNK], SC_DT, tag="kt")
                        nc.vector.tensor_mul(
                            kt, ksb[h][:, ci * CHUNK:(ci + 1) * CHUNK], expNG
                        )
                        k2d = scanp.tile([DH, CHUNK], SC_DT, tag="k2d")
                        nc.vector.tensor_scalar_mul(k2d, kt, expG[:, CHUNK - 1:CHUNK])
                        k2t_ps = psscan.tile([CHUNK, DH], SC_DT, tag="kk")
                        nc.tensor.transpose(k2t_ps, k2d, id_sb)
                        k2t = scanp.tile([CHUNK, DH], SC_DT, tag="k2t")
                        nc.scalar.copy(k2t, k2t_ps)
                        at_ps = psscan.tile([CHUNK, CHUNK], F32, tag="ga")
                        nc.tensor.matmul(at_ps, kt, qt, start=True, stop=True)
                        atm = scanp.tile([CHUNK, CHUNK], SC_DT, tag="atm")
                        nc.vector.tensor_mul(atm, at_ps, u1_sb)
                        v_T = v_tiles[ci][:, h * DH:(h + 1) * DH]
                        pre[(h, ci)] = (qt, k2t, atm, v_T, expG)

                # --- recurrent sweep (chunk-serial per head) ---
                attn_tiles = {}
                for h in range(HPC):
                    attn_t = attnp.tile([DH, SLAB], PW_DT, tag=f"at{h}")
                    attn_tiles[h] = attn_t
                for ci in range(n_tt):
                    for h in range(HPC):
                        bh = b * HPC + h
                        qt, k2t, atm, v_T, expG = pre[(h, ci)]
                        s_old = s_tiles[bh]
                        if SC_DT is not MM_DT:
                            s_mm = scanp.tile([DH, DH], SC_DT, tag=f"sm{bh}")
                            nc.vector.tensor_copy(s_mm, s_old)
                        else:
                            s_mm = s_old
                        ot_ps = psscan.tile([DH, CHUNK], F32, tag="ot")
                        nc.tensor.matmul(ot_ps, s_mm, qt, start=True, stop=False)
                        nc.tensor.matmul(ot_ps, v_T, atm, start=False, stop=True)
                        nc.scalar.copy(
                            attn_tiles[h][:, ci * CHUNK:(ci + 1) * CHUNK], ot_ps
                        )
                        kv_ps = psscan.tile([DH, DH], F32, tag="kk")
                        nc.tensor.matmul(kv_ps, k2t, v_T, start=True, stop=True)
                        s_new = statep.tile([DH, DH], MM_DT, tag=f"S{bh}")
                        nc.vector.scalar_tensor_tensor(
                            s_new, s_old, expG[:, CHUNK - 1:CHUNK], kv_ps,
                            op0=OP.mult, op1=OP.add,
                        )
                        s_tiles[bh] = s_new

                # row-parallel o_proj: out[t, e] += attnT[:, t].T @ woT[:, e]
                for tt in range(n_tt):
                    for eo in range(n_eo):
                        ops = psout.tile([CHUNK, 512], F32, tag="o")
                        for h in range(HPC):
                            nc.tensor.matmul(
                                ops,
                                _mm(attn_tiles[h][:, tt * CHUNK:(tt + 1) * CHUNK]),
                                _mm(wo_sb[:, h, eo * 512:(eo + 1) * 512]),
                                start=(h == 0),
                                stop=(h == HPC - 1),
                            )
                        o_sb = projp.tile([CHUNK, 512], F32, tag="ob")
                        nc.vector.tensor_copy(o_sb, ops)
                        oeng = nc.sync if (slab == nslabs - 1 and (tt + eo) % 2) else nc.gpsimd
                        oeng.dma_start(
                            out[t0 + tt * CHUNK:t0 + (tt + 1) * CHUNK,
                                eo * 512:(eo + 1) * 512],
                            o_sb,
                        )
    nc.compile()
    return nc


_NC_CACHE = {}
LAST_RESULTS = []


def _get_nc(tok, gate_mode):
    key = (tok, gate_mode, MM_DT, PW_DT, SC_DT)
    if key not in _NC_CACHE:
        _NC_CACHE[key] = build_nc(tok, gate_mode)
    return _NC_CACHE[key]


def _make_in_maps(xT, x, Wq, Wk, Wv, Wo, gate_mode, Wg=None, bgv=None, tok=TOK):
    scale = DH ** -0.5
    pw_np = mybir.dt.np(PW_DT)
    u1m = np.triu(np.ones((CHUNK, CHUNK), np.float32))
    idm = np.eye(CHUNK, dtype=np.float32)
    xTp = np.ascontiguousarray(xT.astype(pw_np))
    in_maps = []
    for c in range(NCORES):
        rs = slice(c * DC, (c + 1) * DC)
        m = dict(
            xT=xTp,
            wqT=np.ascontiguousarray((Wq[rs] * scale).T.astype(pw_np)),
            wkT=np.ascontiguousarray(Wk[rs].T.astype(pw_np)),
            wvT=np.ascontiguousarray(Wv[rs].T.astype(pw_np)),
            woT=np.ascontiguousarray(Wo[:, rs].T.astype(pw_np)),
            u1=u1m,
            ident=idm.astype(mybir.dt.np(SC_DT)),
        )
        if gate_mode == "identity":
            m["xcols"] = np.ascontiguousarray(x[:, rs].astype(pw_np))
        else:
            m["wgT"] = np.ascontiguousarray(Wg[rs].T.astype(pw_np))
            m["bg"] = np.ascontiguousarray(bgv[rs].astype(pw_np)).reshape(1, DC)
        in_maps.append(m)
    return in_maps


def _run(nc, in_maps):
    trace = bool(int(os.environ.get("GLA_TRACE", "0")))
    res = run_bass_kernel_spmd(
        nc, in_maps, list(range(NCORES)), trace=trace,
    )
    LAST_RESULTS.append(res)
    total = res.results[0]["out"].astype(np.float32).copy()
    for i in range(1, NCORES):
        total += res.results[i]["out"]
    return total


def kernel(hidden_states, Wq, Wk, Wv, Wo, Wg1, bg1, Wg2, bg2, alpha_list):
    LAST_RESULTS.clear()
    x = np.ascontiguousarray(np.asarray(hidden_states, np.float32).reshape(TOK, HID))
    xT = np.ascontiguousarray(x.T)
    Wq = np.asarray(Wq, np.float32)
    Wk = np.asarray(Wk, np.float32)
    Wv = np.asarray(Wv, np.float32)
    Wo = np.asarray(Wo, np.float32)
    Wg1 = np.asarray(Wg1, np.float32)
    Wg2 = np.asarray(Wg2, np.float32)
    bg1 = np.asarray(bg1, np.float32)
    bg2 = np.asarray(bg2, np.float32)
    al = np.asarray(alpha_list, np.float64)
    a = np.exp(al - al.max())
    a = (a / a.sum()).astype(np.float32)

    gates_equal = np.array_equal(Wg1, Wg2) and np.array_equal(bg1, bg2)
    ident_gate = (
        gates_equal
        and not bg1.any()
        and np.array_equal(Wg1, np.eye(HID, dtype=np.float32))
    )

    if ident_gate:
        nc = _get_nc(TOK, "identity")
        out = _run(nc, _make_in_maps(xT, x, Wq, Wk, Wv, Wo, "identity"))
    elif gates_equal:
        nc = _get_nc(TOK, "general")
        out = _run(nc, _make_in_maps(xT, x, Wq, Wk, Wv, Wo, "general", Wg1, bg1))
    else:
        nc = _get_nc(TOK, "general")
        o1 = _run(nc, _make_in_maps(xT, x, Wq, Wk, Wv, Wo, "general", Wg1, bg1))
        o2 = _run(nc, _make_in_maps(xT, x, Wq, Wk, Wv, Wo, "general", Wg2, bg2))
        out = a[0] * o1 + a[1] * o2

    return out.reshape(B, S, HID)


# revision 31
# speedup vs baseline: 1.2496x; 1.0535x over previous
"""Trainium2 Bass kernel for nn_DualStateLinearAttention.

Reference math (B=2, S=2048, HID=2048, H=16, D=128):
    q = x @ Wq.T, k = x @ Wk.T, v = x @ Wv.T            (split into 16 heads)
    gk_j = clamp(log_sigmoid(x @ Wgj.T + bgj) / 16, min=-50)   j in {1,2}
    o_j  = GLA scan over S with per-key-dim decay exp(gk_j)
    out  = (softmax(alpha)[0] * o1 + softmax(alpha)[1] * o2) @ Wo.T

Strategy (8 NeuronCores, tensor-parallel over heads):
  - 2 heads per core; q/k/v/gate projections column-parallel, o_proj
    row-parallel; each core emits a partial [B*S, HID] output which the
    host sums (the all-reduce of row-parallel o_proj).
  - GLA is evaluated in chunked form (chunk C=128):
        Gc   = inclusive cumsum of g within chunk      (PE matmul w/ triangular ones)
        qt   = q * exp(Gc) * scale   (D-major)
        kt   = k * exp(-Gc)          (D-major)
        AT   = kt.T-contract-qt  -> [ck, cq], masked to ck<=cq
        O^T  = S_prev.T-contract-qt + v.T-contract-ATm     (D-major out)
        K2   = kt * exp(Gc_last)  -> transpose -> [i, dk]
        S    = S_prev * exp(Gc_last) + K2.T-contract-v
  - All matmul operand layouts are chosen so no activation transposes are
    needed (host passes x already transposed); only K2 needs a 128x128 PE
    transpose per chunk.

Host-side dispatch:
  - If Wg1==Wg2 and bg1==bg2 the two GLA branches are identical and the
    softmax weights sum to 1, so a single scan suffices.
  - If additionally Wg1 is the identity with zero bias (the distribution
    this problem ships), the gate projection is skipped entirely and the
    gate input is read directly from x columns.
  - Otherwise the (rare) general path runs the single-gate kernel twice
    and combines on the host (o_proj is linear).
"""

import os
import sys

import numpy as np

for _p in ("/opt/trn_rl_repo",):
    if os.path.isdir(_p) and _p not in sys.path:
        sys.path.insert(0, _p)

import concourse.bass as bass
import concourse.mybir as mybir
import concourse.tile as tile
from concourse import bacc
from concourse.bass_utils import run_bass_kernel_spmd

F32 = mybir.dt.float32
AF = mybir.ActivationFunctionType
OP = mybir.AluOpType

B, S, HID = 2, 2048, 2048
H, DH = 16, 128
NCORES = 8
HPC = H // NCORES          # heads per core
DC = HPC * DH              # per-core head dims (256)
TOK = B * S
SLAB = 512
CHUNK = 128
GATE_NORM = 16.0
CLAMP_MIN = -50.0

# Projection-matmul operand dtype: float32 (exact, 4 cy/row) or float32r
# (~1 cy/row at moving dim >= 256, TF32-like precision).  The scan matmuls
# (moving dim 128) gain nothing from f32r and stay exact fp32.
MM_DT = {"f32": F32, "f32r": mybir.dt.float32r}[os.environ.get("GLA_MM_DT", "f32r")]
# Projection/o_proj operand dtype.  fp16 has the same ~11-bit mantissa as
# TF32/f32r but runs 1 cy/row with fast weight load; safe for the bounded
# projection values (|x|<6, |w|<0.2, |attn|<100).  The scan keeps fp32-range
# dtypes because decayed q/k operands span exp(+-44).
PW_DT = {"f32": F32, "f32r": mybir.dt.float32r, "f16": mybir.dt.float16}[
    os.environ.get("GLA_PW_DT", "f16")]
# Scan matmul operand dtype.  bf16 keeps fp32 exponent range (decayed q/k
# operands span exp(+-40), so fp16 would overflow) at 1 cy/row.  The G
# cumsum operands and the recurrent state stay in MM_DT.
_sc = os.environ.get("GLA_SC_DT", "f16s")
# "f16s": fp16 scan operands with mid-chunk decay shift.  Raw decayed q/k
# span exp(+-44) (fp16 would overflow -> NaN through the mask), but shifting
# q~ by exp(G-Gmid) and k~ by exp(Gmid-G) cancels exactly in q~.T k~ while
# bounding both operands to exp(+-G_half) (realistically e**+-9 * |q,k|,
# far inside fp16 range for gaussian inputs).  The S-side shift folds into
# the s_mm copy; the K2 shift folds into the existing column scale.
SC_SHIFT = _sc == "f16s"
SC_DT = {"bf16": mybir.dt.bfloat16, "f32r": mybir.dt.float32r, "f32": F32,
         "f16s": mybir.dt.float16}[_sc]


def _mm(ap):
    return ap


def build_nc(tok=TOK, gate_mode="identity"):
    """Build the per-core SPMD Bass program.

    gate_mode: "identity" -> gate preactivation is x columns (no projection)
               "general"  -> gate = x @ Wg.T + bg computed on device
    """
    assert tok % SLAB == 0 and (tok // B) % SLAB == 0
    nslabs = tok // SLAB
    slabs_per_b = (tok // B) // SLAB
    n_ct = HID // 128          # contraction tiles
    n_tt = SLAB // CHUNK       # token tiles per slab
    n_eo = HID // 512          # output column tiles

    nc = bacc.Bacc(None, target_bir_lowering=False, debug=False)

    xT = nc.dram_tensor("xT", [HID, tok], PW_DT, kind="ExternalInput")
    wqT = nc.dram_tensor("wqT", [HID, DC], PW_DT, kind="ExternalInput")
    wkT = nc.dram_tensor("wkT", [HID, DC], PW_DT, kind="ExternalInput")
    wvT = nc.dram_tensor("wvT", [HID, DC], PW_DT, kind="ExternalInput")
    woT = nc.dram_tensor("woT", [DC, HID], PW_DT, kind="ExternalInput")
    u1 = nc.dram_tensor("u1", [CHUNK, CHUNK], MM_DT, kind="ExternalInput")
    ident = nc.dram_tensor("ident", [CHUNK, CHUNK], SC_DT, kind="ExternalInput")
    if gate_mode == "identity":
        xcols = nc.dram_tensor("xcols", [tok, DC], PW_DT, kind="ExternalInput")
    else:
        wgT = nc.dram_tensor("wgT", [HID, DC], PW_DT, kind="ExternalInput")
        bg = nc.dram_tensor("bg", [1, DC], PW_DT, kind="ExternalInput")
    out = nc.dram_tensor("out", [tok, HID], F32, kind="ExternalOutput")

    with tile.TileContext(nc) as tc:
        with (
            tc.tile_pool(name="consts", bufs=1) as consts,
            tc.tile_pool(name="xtp", bufs=26) as xtp,
            tc.tile_pool(name="projp", bufs=3) as projp,
            tc.tile_pool(name="tmajp", bufs=8) as tmajp,
            tc.tile_pool(name="tmajt", bufs=10) as tmajt,
            tc.tile_pool(name="gatep", bufs=34) as gatep,
            tc.tile_pool(name="scanp", bufs=10) as scanp,
            tc.tile_pool(name="statep", bufs=2) as statep,
            tc.tile_pool(name="attnp", bufs=2) as attnp,
            tc.tile_pool(name="ps_proj", bufs=2, space=bass.MemorySpace.PSUM) as psproj,
            tc.tile_pool(name="ps_scan", bufs=1, space=bass.MemorySpace.PSUM) as psscan,
            tc.tile_pool(name="ps_out", bufs=2, space=bass.MemorySpace.PSUM) as psout,
        ):
            wq_sb = consts.tile([128, n_ct, DC], PW_DT)
            wk_sb = consts.tile([128, n_ct, DC], PW_DT)
            wv_sb = consts.tile([128, n_ct, DC], PW_DT)
            wo_sb = consts.tile([128, HPC, HID], PW_DT)
            for ct in range(n_ct):
                cs = slice(ct * 128, (ct + 1) * 128)
                nc.scalar.dma_start(wq_sb[:, ct, :], wqT[cs, :])
                nc.scalar.dma_start(wk_sb[:, ct, :], wkT[cs, :])
            u1_sb = consts.tile([CHUNK, CHUNK], MM_DT)
            nc.scalar.dma_start(u1_sb, u1[:, :])
            id_sb = consts.tile([CHUNK, CHUNK], SC_DT)
            nc.scalar.dma_start(id_sb, ident[:, :])
            for ct in range(n_ct):
                cs = slice(ct * 128, (ct + 1) * 128)
                nc.gpsimd.dma_start(wv_sb[:, ct, :], wvT[cs, :])
            nc.gpsimd.dma_start(
                wo_sb, woT[:, :].rearrange("(hp p) e -> p hp e", p=128)
            )
            if gate_mode == "general":
                wg_sb = consts.tile([128, n_ct, DC], PW_DT)
                nc.sync.dma_start(wg_sb, wgT[:, :].rearrange("(ct p) d -> p ct d", p=128))
                bg_sb = consts.tile([1, DC], PW_DT)
                nc.sync.dma_start(bg_sb, bg[:, :])
                ones1 = consts.tile([1, CHUNK], PW_DT)
                nc.vector.memset(ones1, 1.0)

            # per-(batch, head) recurrent state [dk, dv]
            s_tiles = {}
            for bh in range(B * HPC):
                t = statep.tile([DH, DH], MM_DT, tag=f"S{bh}")
                nc.vector.memset(t.bitcast(F32), 0.0)
                s_tiles[bh] = t

            # identity-gate mode: all gates depend only on x columns, so
            # compute every chunk's gate up front with exactly one
            # exp-batch and one ln-batch (2 ACT table loads total).
            xts0 = []
            for ct in range(n_ct):
                xt = xtp.tile([128, SLAB], PW_DT, tag="xt")
                nc.sync.dma_start(xt, xT[ct * 128:(ct + 1) * 128, 0:SLAB])
                xts0.append(xt)

            all_g_tiles = []
            if gate_mode == "identity":
                n_gt = tok // CHUNK
                for g0 in range(0, n_gt, 8):
                    gsps = []
                    for gi in range(g0, g0 + 8):
                        gsrc = tmajt.tile([128, DC], PW_DT, tag="gx")
                        nc.sync.dma_start(
                            gsrc, xcols[gi * CHUNK:(gi + 1) * CHUNK, :]
                        )
                        gsp = tmajt.tile([128, DC], F32, tag="gsp")
                        nc.scalar.activation(gsp, gsrc, AF.Exp, scale=-1.0)
                        gsps.append(gsp)
                    for gj, gi in enumerate(range(g0, g0 + 8)):
                        # log_sigmoid(z) = -ln(1 + exp(-z)); clamped /GATE_NORM
                        lns = tmajt.tile([128, DC], F32, tag="lns")
                        nc.scalar.activation(lns, gsps[gj], AF.Ln, bias=1.0)
                        g_sb = gatep.tile([128, DC], MM_DT, tag="g")
                        nc.vector.tensor_scalar(
                            g_sb, lns, -1.0 / GATE_NORM, CLAMP_MIN,
                            op0=OP.mult, op1=OP.max,
                        )
                        all_g_tiles.append(g_sb)

            for slab in range(nslabs):
                b = slab // slabs_per_b
                t0 = slab * SLAB

                if slab == 0:
                    xts = xts0
                else:
                    xts = []
                    for ct in range(n_ct):
                        xt = xtp.tile([128, SLAB], PW_DT, tag="xt")
                        nc.sync.dma_start(
                            xt, xT[ct * 128:(ct + 1) * 128, t0:t0 + SLAB]
                        )
                        xts.append(xt)

                # D-major projections: q, k  (out [d, t])
                qsb, ksb = {}, {}
                for h in range(HPC):
                    for name, wsb, store in (("q", wq_sb, qsb), ("k", wk_sb, ksb)):
                        ps = psproj.tile([128, SLAB], F32, tag="pp")
                        for ct in range(n_ct):
                            nc.tensor.matmul(
                                ps,
                                _mm(wsb[:, ct, h * DH:(h + 1) * DH]),
                                _mm(xts[ct]),
                                start=(ct == 0),
                                stop=(ct == n_ct - 1),
                            )
                        sbt = projp.tile([128, SLAB], F32, tag=name)
                        nc.vector.tensor_copy(sbt, ps)
                        store[h] = sbt

                # T-major projections: v (and gate preact z), out [t, d]
                v_tiles, g_tiles = [], []
                for tt in range(n_tt):
                    ps = psproj.tile([128, DC], F32, tag="pp")
                    for ct in range(n_ct):
                        nc.tensor.matmul(
                            ps,
                            _mm(xts[ct][:, tt * CHUNK:(tt + 1) * CHUNK]),
                            _mm(wv_sb[:, ct, :]),
                            start=(ct == 0),
                            stop=(ct == n_ct - 1),
                        )
                    v_sb = tmajp.tile([128, DC], SC_DT, tag="v")
                    nc.vector.tensor_copy(v_sb, ps)
                    v_tiles.append(v_sb)


                if gate_mode == "identity":
                    g_tiles = all_g_tiles[slab * n_tt:(slab + 1) * n_tt]
                else:
                    # gate projection z = x @ Wg.T + bg, then exp/ln batched
                    sps = []
                    for tt in range(n_tt):
                        zps = psproj.tile([128, DC], F32, tag="pp")
                        for ct in range(n_ct):
                            nc.tensor.matmul(
                                zps,
                                _mm(xts[ct][:, tt * CHUNK:(tt + 1) * CHUNK]),
                                _mm(wg_sb[:, ct, :]),
                                start=(ct == 0),
                                stop=False,
                            )
                        nc.tensor.matmul(zps, _mm(ones1), _mm(bg_sb), start=False, stop=True)
                        sp = tmajt.tile([128, DC], F32, tag="gsp")
                        nc.scalar.activation(sp, zps, AF.Exp, scale=-1.0)
                        sps.append(sp)
                    for tt in range(n_tt):
                        lns = tmajt.tile([128, DC], F32, tag="lns")
                        nc.scalar.activation(lns, sps[tt], AF.Ln, bias=1.0)
                        g_sb = tmajp.tile([128, DC], MM_DT, tag="g")
                        nc.vector.tensor_scalar(
                            g_sb, lns, -1.0 / GATE_NORM, CLAMP_MIN,
                            op0=OP.mult, op1=OP.max,
                        )
                        g_tiles.append(g_sb)

                # --- batched scan prologue (off the recurrent chain) ---
                # G matmuls + exps + decay muls + AT for all (head, chunk)
                # pairs, emitted function-batched so the ACT engine does not
                # thrash its activation table.
                pre = {}
                for h in range(HPC):
                    for ci in range(n_tt):
                        g_T = g_tiles[ci][:, h * DH:(h + 1) * DH]
                        gd_ps = psscan.tile([DH, CHUNK], F32, tag="gg")
                        nc.tensor.matmul(gd_ps, g_T, u1_sb, start=True, stop=True)
                        expG = scanp.tile([DH, CHUNK], F32, tag="eg")
                        nc.scalar.activation(expG, gd_ps, AF.Exp)
                        expNG = scanp.tile([DH, CHUNK], F32, tag="eng")
                        nc.scalar.activation(expNG, gd_ps, AF.Exp, scale=-1.0)
                        qt = scanp.tile([DH, CHUNK], SC_DT, tag="qt")
                        kt = scanp.tile([DH, CHUNK], SC_DT, tag="kt")
                        k2d = scanp.tile([DH, CHUNK], SC_DT, tag="k2d")
                        if SC_SHIFT:
                            mid = CHUNK // 2
                            egmid = expG[:, mid - 1:mid]
                            engmid = expNG[:, mid - 1:mid]
                            # qt = q * exp(G - Gmid), kt = k * exp(Gmid - G)
                            nc.vector.scalar_tensor_tensor(
                                qt, qsb[h][:, ci * CHUNK:(ci + 1) * CHUNK],
                                engmid, expG, op0=OP.mult, op1=OP.mult,
                            )
                            nc.vector.scalar_tensor_tensor(
                                kt, ksb[h][:, ci * CHUNK:(ci + 1) * CHUNK],
                                egmid, expNG, op0=OP.mult, op1=OP.mult,
                            )
                            # K2 = kt * exp(G_last - Gmid)
                            c2 = scanp.tile([DH, 1], F32, tag="c2")
                            nc.vector.tensor_mul(
                                c2, expG[:, CHUNK - 1:CHUNK], engmid
                            )
                            nc.vector.tensor_scalar_mul(k2d, kt, c2)
                        else:
                            egmid = None
                            nc.vector.tensor_mul(
                                qt, qsb[h][:, ci * CHUNK:(ci + 1) * CHUNK], expG
                            )
                            nc.vector.tensor_mul(
                                kt, ksb[h][:, ci * CHUNK:(ci + 1) * CHUNK], expNG
                            )
                            nc.vector.tensor_scalar_mul(
                                k2d, kt, expG[:, CHUNK - 1:CHUNK]
                            )
                        k2t_ps = psscan.tile([CHUNK, DH], SC_DT, tag="kk")
                        nc.tensor.transpose(k2t_ps, k2d, id_sb)
                        k2t = scanp.tile([CHUNK, DH], SC_DT, tag="k2t")
                        nc.scalar.copy(k2t, k2t_ps)
                        at_ps = psscan.tile([CHUNK, CHUNK], F32, tag="ga")
                        nc.tensor.matmul(at_ps, kt, qt, start=True, stop=True)
                        atm = scanp.tile([CHUNK, CHUNK], SC_DT, tag="atm")
                        nc.vector.tensor_mul(atm, at_ps, u1_sb)
                        v_T = v_tiles[ci][:, h * DH:(h + 1) * DH]
                        pre[(h, ci)] = (qt, k2t, atm, v_T, expG, egmid)

                # --- recurrent sweep (chunk-serial per head) ---
                attn_tiles = {}
                for h in range(HPC):
                    attn_t = attnp.tile([DH, SLAB], PW_DT, tag=f"at{h}")
                    attn_tiles[h] = attn_t
                for ci in range(n_tt):
                    for h in range(HPC):
                        bh = b * HPC + h
                        qt, k2t, atm, v_T, expG, egmid = pre[(h, ci)]
                        s_old = s_tiles[bh]
                        if SC_SHIFT:
                            # fold the q-side shift into S: s_mm = S * exp(Gmid)
                            s_mm = scanp.tile([DH, DH], SC_DT, tag=f"sm{bh}")
                            nc.vector.tensor_scalar_mul(s_mm, s_old, egmid)
                        elif SC_DT is not MM_DT:
                            s_mm = scanp.tile([DH, DH], SC_DT, tag=f"sm{bh}")
                            nc.vector.tensor_copy(s_mm, s_old)
                        else:
                            s_mm = s_old
                        ot_ps = psscan.tile([DH, CHUNK], F32, tag="ot")
                        nc.tensor.matmul(ot_ps, s_mm, qt, start=True, stop=False)
                        nc.tensor.matmul(ot_ps, v_T, atm, start=False, stop=True)
                        nc.scalar.copy(
                            attn_tiles[h][:, ci * CHUNK:(ci + 1) * CHUNK], ot_ps
                        )
                        kv_ps = psscan.tile([DH, DH], F32, tag="kk")
                        nc.tensor.matmul(kv_ps, k2t, v_T, start=True, stop=True)
                        s_new = statep.tile([DH, DH], MM_DT, tag=f"S{bh}")
                        nc.vector.scalar_tensor_tensor(
                            s_new, s_old, expG[:, CHUNK - 1:CHUNK], kv_ps,
                            op0=OP.mult, op1=OP.add,
                        )
                        s_tiles[bh] = s_new

                # row-parallel o_proj: out[t, e] += attnT[:, t].T @ woT[:, e]
                for tt in range(n_tt):
                    for eo in range(n_eo):
                        ops = psout.tile([CHUNK, 512], F32, tag="o")
                        for h in range(HPC):
                            nc.tensor.matmul(
                                ops,
                                _mm(attn_tiles[h][:, tt * CHUNK:(tt + 1) * CHUNK]),
                                _mm(wo_sb[:, h, eo * 512:(eo + 1) * 512]),
                                start=(h == 0),
                                stop=(h == HPC - 1),
                            )
                        o_sb = projp.tile([CHUNK, 512], F32, tag="ob")
                        nc.vector.tensor_copy(o_sb, ops)
                        oeng = nc.sync if (slab == nslabs - 1 and (tt + eo) % 2) else nc.gpsimd
                        oeng.dma_start(
                            out[t0 + tt * CHUNK:t0 + (tt + 1) * CHUNK,
                                eo * 512:(eo + 1) * 512],
                            o_sb,
                        )
    nc.compile()
    return nc


_NC_CACHE = {}
LAST_RESULTS = []


def _get_nc(tok, gate_mode):
    key = (tok, gate_mode, MM_DT, PW_DT, SC_DT)
    if key not in _NC_CACHE:
        _NC_CACHE[key] = build_nc(tok, gate_mode)
    return _NC_CACHE[key]


def _make_in_maps(xT, x, Wq, Wk, Wv, Wo, gate_mode, Wg=None, bgv=None, tok=TOK):
    scale = DH ** -0.5
    pw_np = mybir.dt.np(PW_DT)
    u1m = np.triu(np.ones((CHUNK, CHUNK), np.float32))
    idm = np.eye(CHUNK, dtype=np.float32)
    xTp = np.ascontiguousarray(xT.astype(pw_np))
    in_maps = []
    for c in range(NCORES):
        rs = slice(c * DC, (c + 1) * DC)
        m = dict(
            xT=xTp,
            wqT=np.ascontiguousarray((Wq[rs] * scale).T.astype(pw_np)),
            wkT=np.ascontiguousarray(Wk[rs].T.astype(pw_np)),
            wvT=np.ascontiguousarray(Wv[rs].T.astype(pw_np)),
            woT=np.ascontiguousarray(Wo[:, rs].T.astype(pw_np)),
            u1=u1m,
            ident=idm.astype(mybir.dt.np(SC_DT)),
        )
        if gate_mode == "identity":
            m["xcols"] = np.ascontiguousarray(x[:, rs].astype(pw_np))
        else:
            m["wgT"] = np.ascontiguousarray(Wg[rs].T.astype(pw_np))
            m["bg"] = np.ascontiguousarray(bgv[rs].astype(pw_np)).reshape(1, DC)
        in_maps.append(m)
    return in_maps


def _run(nc, in_maps):
    trace = bool(int(os.environ.get("GLA_TRACE", "0")))
    res = run_bass_kernel_spmd(
        nc, in_maps, list(range(NCORES)), trace=trace,
    )
    LAST_RESULTS.append(res)
    total = res.results[0]["out"].astype(np.float32).copy()
    for i in range(1, NCORES):
        total += res.results[i]["out"]
    return total


def kernel(hidden_states, Wq, Wk, Wv, Wo, Wg1, bg1, Wg2, bg2, alpha_list):
    LAST_RESULTS.clear()
    x = np.ascontiguousarray(np.asarray(hidden_states, np.float32).reshape(TOK, HID))
    xT = np.ascontiguousarray(x.T)
    Wq = np.asarray(Wq, np.float32)
    Wk = np.asarray(Wk, np.float32)
    Wv = np.asarray(Wv, np.float32)
    Wo = np.asarray(Wo, np.float32)
    Wg1 = np.asarray(Wg1, np.float32)
    Wg2 = np.asarray(Wg2, np.float32)
    bg1 = np.asarray(bg1, np.float32)
    bg2 = np.asarray(bg2, np.float32)
    al = np.asarray(alpha_list, np.float64)
    a = np.exp(al - al.max())
    a = (a / a.sum()).astype(np.float32)

    gates_equal = np.array_equal(Wg1, Wg2) and np.array_equal(bg1, bg2)
    ident_gate = (
        gates_equal
        and not bg1.any()
        and np.array_equal(Wg1, np.eye(HID, dtype=np.float32))
    )

    if ident_gate:
        nc = _get_nc(TOK, "identity")
        out = _run(nc, _make_in_maps(xT, x, Wq, Wk, Wv, Wo, "identity"))
    elif gates_equal:
        nc = _get_nc(TOK, "general")
        out = _run(nc, _make_in_maps(xT, x, Wq, Wk, Wv, Wo, "general", Wg1, bg1))
    else:
        nc = _get_nc(TOK, "general")
        o1 = _run(nc, _make_in_maps(xT, x, Wq, Wk, Wv, Wo, "general", Wg1, bg1))
        o2 = _run(nc, _make_in_maps(xT, x, Wq, Wk, Wv, Wo, "general", Wg2, bg2))
        out = a[0] * o1 + a[1] * o2

    return out.reshape(B, S, HID)
